# revision 26
# baseline (speedup 1.0000x reference)
"""DeepseekV2 MLA attention on 8 Trainium2 NeuronCores.

Sharding (uniform SPMD, no control divergence):
- A-projection, q-up-projection, final W_O: token-sharded (core c owns
  tokens [256c, 256c+256)).
- Attention (absorbed MLA over the compressed KV latent): head-sharded
  (core c owns heads {2c, 2c+1}).
- Three collectives connect the shardings: AllGather of the kv latent
  (feature-major, bf16), AllToAll of q^T (token->head resharding),
  AllToAll of normalized o^T (head->token resharding).

All matmuls run in bf16 with fp32 PSUM accumulation. RMSNorm weights are
folded into the adjacent weight matrices on the host. Softmax runs
unnormalized (logits are small by construction: std ~0.7) with the
denominator obtained by appending a ones-column to V; normalization is a
per-partition scale on the token-major attention output.

Weight matrices are repacked on the host into tiles whose SBUF-partition
rows are 3-16KB contiguous in DRAM so each DMA descriptor moves large
packets (the DMA ring round-robins packets over 16 engines; throughput
per engine is packet-size-bound). W_O tiles are prefetched during the
attention phase, when the DMA engines are otherwise idle.
"""

import os
import sys

for _p in ("/opt/trn_rl_repo", "/root/.axon_site", "/root/.axon_site/_ro/trn_rl_repo",
           "/root/.axon_site/_ro/pypackages"):
    if os.path.isdir(_p) and _p not in sys.path:
        sys.path.insert(0, _p)

import numpy as np
import ml_dtypes

import concourse.bass as bass
import concourse.tile as tile
from concourse import bacc, mybir
from concourse.bass_utils import run_bass_kernel_spmd
from concourse.masks import make_identity

# Problem constants (hardcoded per contract)
T, HID, H = 2048, 5120, 16
DN, DR, DV = 128, 64, 128
QL, KVL = 1536, 512
EPS = 1e-6
THETA = 10000.0
SCALE = (DN + DR) ** -0.5

NCORES = 8
TLOC = T // NCORES          # 256 tokens per core
HLOC = H // NCORES          # 2 heads per core
MCH = TLOC // 128           # 2 token chunks of 128
KD = HID // 128             # 40 contraction chunks for A-proj
QKD = QL // 128             # 12 contraction chunks for q-up
LC = KVL // 128             # 4 latent chunks
NQB = T // 128              # 16 query/key blocks of 128
DQK = DN + DR               # 192
DVE_ = DV + 1               # 129: extra ones-column for softmax denominator
KVPE = KVL + DR             # 576

WF_KV_TILES = [1, 1, 2] + [4] * 9   # k-chunks per wf_kv tile (sum 40)
NT_Q = 20                   # wf q tiles (2 k-chunks each)
NHT = HID // 512            # 10 W_O output groups

BF = mybir.dt.bfloat16
F32 = mybir.dt.float32

_NC_CACHE = None
_last_in_maps = None


def _rope_pair(nc, pool, x_pairs, cos, sin, out_pairs, shape):
    """Interleaved rope: out1 = x1*cos - x2*sin ; out2 = x2*cos + x1*sin."""
    x1, x2 = x_pairs[:, 0], x_pairs[:, 1]
    o1, o2 = out_pairs[:, 0], out_pairs[:, 1]
    tm1 = pool.tile([128] + shape, F32, tag="rope_tm1", name="rope_tm1")
    tm2 = pool.tile([128] + shape, F32, tag="rope_tm2", name="rope_tm2")
    tm3 = pool.tile([128] + shape, F32, tag="rope_tm3", name="rope_tm3")
    nc.vector.tensor_mul(tm1[:], x1, cos)
    nc.vector.tensor_mul(tm2[:], x2, sin)
    nc.vector.tensor_mul(tm3[:], x1, sin)
    nc.vector.tensor_sub(o1, tm1[:], tm2[:])
    nc.vector.tensor_mul(tm1[:], x2, cos)
    nc.vector.tensor_add(o2, tm1[:], tm3[:])


def build_nc():
    nc = bacc.Bacc("TRN2", target_bir_lowering=False, debug=False,
                   num_devices=NCORES)

    # host-repacked inputs; rows (last axis) are DRAM-contiguous per tile
    hT = nc.dram_tensor("hT", [128, KD * TLOC], BF, kind="ExternalInput")
    wf_kv = nc.dram_tensor("wf_kv", [KD * 128 * KVPE], BF,
                           kind="ExternalInput")
    wf_q = nc.dram_tensor("wf_q", [NT_Q, 128, 2 * QL], BF, kind="ExternalInput")
    wqb = nc.dram_tensor("wqb", [2, QKD, 128, QL], BF, kind="ExternalInput")
    cs = nc.dram_tensor("cs", [TLOC, DR], F32, kind="ExternalInput")
    wkcT = nc.dram_tensor("wkcT", [HLOC, DN, KVL], BF, kind="ExternalInput")
    wvc = nc.dram_tensor("wvc", [KVL, HLOC * DV], BF, kind="ExternalInput")
    wo = nc.dram_tensor("wo", [2 * NHT, 128, (H // 2) * 512], BF,
                        kind="ExternalInput")
    cmask = nc.dram_tensor("cmask", [128, HLOC, 128], BF, kind="ExternalInput")
    out = nc.dram_tensor("out", [TLOC, HID], F32, kind="ExternalOutput")

    RG = [list(range(NCORES))]

    with tile.TileContext(nc) as tc:
        consts_cm = tc.tile_pool(name="consts", bufs=1)
        consts = consts_cm.__enter__()
        dram_cm = tc.tile_pool(name="dram", bufs=1, space="DRAM")
        dram = dram_cm.__enter__()
        wstream_cm = tc.tile_pool(name="wstream", bufs=4)
        wstream = wstream_cm.__enter__()
        ps_mm_cm = tc.tile_pool(name="ps_mm", bufs=4, space="PSUM")
        ps_mm = ps_mm_cm.__enter__()
        ps_tr_cm = tc.tile_pool(name="ps_tr", bufs=2, space="PSUM")
        ps_tr = ps_tr_cm.__enter__()
        # W_O prefetch pool: opened before `early` so its bytes are never
        # reused from phase-1 tiles (no false WAR delaying the prefetch).
        wo_cm = tc.tile_pool(name="wo_pool", bufs=3)
        wo_pool = wo_cm.__enter__()
        ps_x_cm = tc.tile_pool(name="ps_x", bufs=2, space="PSUM")
        ps_x = ps_x_cm.__enter__()

        ident = consts.tile([128, 128], BF, name="ident")
        make_identity(nc, ident[:])
        cmask_sb = consts.tile([128, HLOC, 128], BF, name="cmask_sb")
        wkc_sb = consts.tile([128, HLOC, KVL], BF, name="wkc_sb")
        eps_sb = consts.tile([128, 1], F32, name="eps_sb")
        nc.vector.memset(eps_sb[:], float(EPS))
        wvc_sb = consts.tile([128, LC, HLOC * DV], BF, name="wvc_sb")

        # collective DRAM tiles
        ag_in = dram.tile([KVPE, TLOC], BF, name="ag_in")
        ag_out = dram.tile([NCORES, KVPE, TLOC], BF, addr_space="Shared",
                           name="ag_out")
        a2aq_in = dram.tile([NCORES, HLOC, DQK, TLOC], BF, name="a2aq_in")
        a2aq_out = dram.tile([NCORES, HLOC, DQK, TLOC], BF, name="a2aq_out")
        a2ao_in = dram.tile([NCORES, HLOC, DV, TLOC], BF, name="a2ao_in")
        a2ao_out = dram.tile([NCORES, HLOC, DV, TLOC], BF, name="a2ao_out")

        # ---------------- Phase 1: token-sharded projections ---------------
        early_cm = tc.tile_pool(name="early", bufs=1)
        early = early_cm.__enter__()
        tmp_cm = tc.tile_pool(name="tmp", bufs=1)
        tmp = tmp_cm.__enter__()

        hT_sb = early.tile([128, KD * TLOC], BF, name="hT_sb")

        def _hT_load(k0, nk):
            nc.sync.dma_start(out=hT_sb[:, k0 * TLOC:(k0 + nk) * TLOC],
                              in_=hT[:, k0 * TLOC:(k0 + nk) * TLOC])

        # first A-proj matmuls gate on a small slice; big slices interleave
        # with the wf_kv tile stream (issued inside the loop below)
        _hT_load(0, 2)
        _hT_load(2, 6)
        cs_sb = early.tile([128, MCH, DR], F32, name="cs_sb")
        nc.sync.dma_start(out=cs_sb[:],
                          in_=cs[:, :].rearrange("(m p) d -> p m d", p=128))

        qkv_sb = early.tile([128, MCH, QL + KVPE], F32, name="qkv_sb")

        def hT_lhs(k, m):
            return hT_sb[:, k * TLOC + m * 128: k * TLOC + m * 128 + 128]

        # --- A-proj, kv+pe columns (one fused pass over k) ---
        pss_kv = [ps_mm.tile([128, KVL], F32, tag="mm", name="aproj_kv")
                  for _ in range(MCH)]
        pss_pe = [ps_x.tile([128, DR], F32, tag="x", name="aproj_pe")
                  for _ in range(MCH)]
        _off = 0
        _k0 = 0
        for nk in WF_KV_TILES:
            wt = wstream.tile([128, 4 * KVPE], BF, tag="wf_kv", bufs=3,
                              name="wf_kv_t")
            nc.sync.dma_start(
                out=wt[:, :nk * KVPE],
                in_=wf_kv[_off:_off + 128 * nk * KVPE]
                    .rearrange("(p w) -> p w", p=128))
            _off += 128 * nk * KVPE
            if _k0 == 4:
                _hT_load(8, 16)
            elif _k0 == 16:
                _hT_load(24, 16)
            for kk in range(nk):
                k = _k0 + kk
                for m in range(MCH):
                    nc.tensor.matmul(pss_kv[m][:], hT_lhs(k, m),
                                     wt[:, kk * KVPE: kk * KVPE + KVL],
                                     start=(k == 0), stop=(k == KD - 1))
                    nc.tensor.matmul(pss_pe[m][:], hT_lhs(k, m),
                                     wt[:, kk * KVPE + KVL: (kk + 1) * KVPE],
                                     start=(k == 0), stop=(k == KD - 1))
            _k0 += nk
        for m in range(MCH):
            nc.vector.tensor_copy(qkv_sb[:, m, QL:QL + KVL], pss_kv[m][:])
            nc.scalar.copy(qkv_sb[:, m, QL + KVL:], pss_pe[m][:])

        # --- kv latent rmsnorm + rope(k_pe) + AllGather -------------------
        kvlat_bf = early.tile([128, MCH, KVL], BF, name="kvlat_bf")
        kpe_bf = early.tile([128, MCH, DR], BF, name="kpe_bf")
        agin_sb = early.tile([128, LC, MCH, 128], BF, name="agin_sb")
        agpe_sb = early.tile([64, MCH, 128], BF, name="agpe_sb")

        for m in range(MCH):
            sq = tmp.tile([128, KVL], F32, tag="sq_kv", name="sq_kv")
            ssum = tmp.tile([128, 1], F32, tag="ssum_kv", name="ssum_kv")
            nc.scalar.activation(sq[:], qkv_sb[:, m, QL:QL + KVL],
                                 mybir.ActivationFunctionType.Square,
                                 accum_out=ssum[:])
            rstd = tmp.tile([128, 1], F32, tag="rstd_kv", name="rstd_kv")
            nc.scalar.activation(rstd[:], ssum[:],
                                 mybir.ActivationFunctionType.Sqrt,
                                 bias=eps_sb[:], scale=1.0 / KVL)
            rinv = tmp.tile([128, 1], F32, tag="rinv_kv", name="rinv_kv")
            nc.vector.reciprocal(rinv[:], rstd[:])
            nc.vector.tensor_scalar_mul(kvlat_bf[:, m], in0=qkv_sb[:, m, QL:QL + KVL],
                                        scalar1=rinv[:])
            kv_pairs = qkv_sb[:, m, QL + KVL:].rearrange("p (i two) -> p two i", two=2)
            out_pairs = kpe_bf[:, m].rearrange("p (i two) -> p two i", two=2)
            _rope_pair(nc, tmp, kv_pairs,
                       cs_sb[:, m, :DR // 2], cs_sb[:, m, DR // 2:],
                       out_pairs, [DR // 2])
            for lc in range(LC):
                pt = ps_tr.tile([128, 128], BF, tag="tr", name="pt_tr")
                nc.tensor.transpose(pt[:], kvlat_bf[:, m, lc * 128:(lc + 1) * 128],
                                    ident[:])
                nc.vector.tensor_copy(agin_sb[:, lc, m, :], pt[:])
            ptp = ps_tr.tile([64, 128], BF, tag="tr", name="ptp_tr")
            nc.tensor.transpose(ptp[:], kpe_bf[:, m], ident[:])
            nc.vector.tensor_copy(agpe_sb[:, m, :], ptp[:])

        nc.gpsimd.dma_start(
            out=ag_in[:KVL, :].rearrange("(c p) m -> p c m", p=128)
                              .rearrange("p c (m t) -> p c m t", m=MCH),
            in_=agin_sb[:])
        nc.gpsimd.dma_start(
            out=ag_in[KVL:, :].rearrange("p (m t) -> p m t", m=MCH),
            in_=agpe_sb[:])
        nc.gpsimd.collective_compute(
            "AllGather", mybir.AluOpType.bypass, replica_groups=RG,
            ins=[ag_in.opt()], outs=[ag_out.opt()])

        # attention-phase consts: small loads tucked behind the q A-proj
        nc.sync.dma_start(out=wvc_sb[:],
                          in_=wvc[:, :].rearrange("(c p) v -> p c v", p=128))
        nc.sync.dma_start(out=wkc_sb[:], in_=wkcT[:, :, :].rearrange("h d l -> d h l"))
        nc.sync.dma_start(out=cmask_sb[:], in_=cmask[:, :, :])

        # --- A-proj, q columns (one fused pass over k) --------------------
        pss_q = [[None] * 3 for _ in range(MCH)]
        for m in range(MCH):
            for g in range(3):
                if m == 1 and g == 2:
                    pss_q[m][g] = ps_x.tile([128, 512], F32, tag="x",
                                            name="aproj_qx")
                elif m == 0 and g == 2:
                    pss_q[m][g] = ps_x.tile([128, 512], F32, tag="x",
                                            name="aproj_qx")
                else:
                    pss_q[m][g] = ps_mm.tile([128, 512], F32, tag="mm",
                                             name="aproj_q")
        for t in range(NT_Q):
            wt = wstream.tile([128, 2 * QL], BF, tag="wf_q", bufs=3,
                              name="wf_q_t")
            nc.sync.dma_start(out=wt[:], in_=wf_q[t])
            for kk in range(2):
                k = 2 * t + kk
                for m in range(MCH):
                    for g in range(3):
                        nc.tensor.matmul(
                            pss_q[m][g][:], hT_lhs(k, m),
                            wt[:, kk * QL + g * 512: kk * QL + (g + 1) * 512],
                            start=(k == 0), stop=(k == KD - 1))
        for m in range(MCH):
            for g in range(3):
                if g % 2 == 0:
                    nc.scalar.copy(qkv_sb[:, m, g * 512:(g + 1) * 512],
                                   pss_q[m][g][:])
                else:
                    nc.vector.tensor_copy(qkv_sb[:, m, g * 512:(g + 1) * 512],
                                          pss_q[m][g][:])

        # --- q rmsnorm + transpose --------------------------------------
        qan_bf = early.tile([128, MCH, QL], BF, name="qan_bf")
        for m in range(MCH):
            sq2 = tmp.tile([128, QL], F32, tag="sq_q", name="sq_q")
            ssum2 = tmp.tile([128, 1], F32, tag="ssum_q", name="ssum_q")
            nc.scalar.activation(sq2[:], qkv_sb[:, m, :QL],
                                 mybir.ActivationFunctionType.Square,
                                 accum_out=ssum2[:])
            rstd2 = tmp.tile([128, 1], F32, tag="rstd_q", name="rstd_q")
            nc.scalar.activation(rstd2[:], ssum2[:],
                                 mybir.ActivationFunctionType.Sqrt,
                                 bias=eps_sb[:], scale=1.0 / QL)
            rinv2 = tmp.tile([128, 1], F32, tag="rinv_q", name="rinv_q")
            nc.vector.reciprocal(rinv2[:], rstd2[:])
            nc.vector.tensor_scalar_mul(qan_bf[:, m], in0=qkv_sb[:, m, :QL],
                                        scalar1=rinv2[:])

        qanT_sb = early.tile([128, QKD, TLOC], BF, name="qanT_sb")
        for kc in range(QKD):
            for m in range(MCH):
                pt = ps_tr.tile([128, 128], BF, tag="tr", name="pt_tr")
                nc.tensor.transpose(pt[:], qan_bf[:, m, kc * 128:(kc + 1) * 128],
                                    ident[:])
                nc.vector.tensor_copy(qanT_sb[:, kc, m * 128:(m + 1) * 128], pt[:])

        # --- q-up projection: two column halves over cached qanT ----------
        q_bf = early.tile([128, MCH, H, DQK], BF, name="q_bf")
        for half in range(2):
            psq = [[None] * 3 for _ in range(MCH)]
            for m in range(MCH):
                for g in range(3):
                    if g == 2:
                        psq[m][g] = ps_x.tile([128, 512], F32, tag="x",
                                              name="qup_psx")
                    else:
                        psq[m][g] = ps_mm.tile([128, 512], F32, tag="mm",
                                               name="qup_ps")
            for kc in range(QKD):
                wqb_t = wstream.tile([128, QL], BF, tag="wqb_t", bufs=4,
                                     name="wqb_t")
                nc.sync.dma_start(out=wqb_t[:], in_=wqb[half, kc])
                for m in range(MCH):
                    for g in range(3):
                        nc.tensor.matmul(
                            psq[m][g][:], qanT_sb[:, kc, m * 128:(m + 1) * 128],
                            wqb_t[:, g * 512:(g + 1) * 512],
                            start=(kc == 0), stop=(kc == QKD - 1))
            for m in range(MCH):
                dst = q_bf[:, m].rearrange("p h d -> p (h d)")
                for g in range(3):
                    cols = slice(half * QL + g * 512, half * QL + (g + 1) * 512)
                    if g % 2 == 0:
                        nc.scalar.copy(dst[:, cols], psq[m][g][:])
                    else:
                        nc.vector.tensor_copy(dst[:, cols], psq[m][g][:])

        # --- rope on q_pe (in place, bf16) --------------------------------
        crep = early.tile([128, MCH, H, DR], F32, name="crep")
        _cs_base = cs[:, :]
        for m in range(MCH):
            nc.sync.dma_start(
                out=crep[:, m],
                in_=bass.AP(tensor=_cs_base.tensor, offset=m * 128 * DR,
                            ap=[[DR, 128], [0, H], [1, DR]]))
        for m in range(MCH):
            q_pairs = q_bf[:, m, :, DN:].rearrange("p h (i two) -> p two h i", two=2)
            _rope_pair(nc, tmp, q_pairs,
                       crep[:, m, :, :DR // 2], crep[:, m, :, DR // 2:],
                       q_pairs, [H, DR // 2])

        # --- transpose q and AllToAll (token -> head resharding) ----------
        aq0_sb = early.tile([128, H, MCH, 128], BF, name="aq0_sb")
        aq1_sb = early.tile([64, H, MCH, 128], BF, name="aq1_sb")
        _aqv = a2aq_in[:, :, :, :].rearrange("s hh d t -> (s hh) d t") \
                                  .rearrange("h d (m t) -> d h m t", m=MCH)
        for h in range(H):
            for m in range(MCH):
                pt0 = ps_tr.tile([128, 128], BF, tag="tr", name="pt0_tr")
                nc.tensor.transpose(pt0[:], q_bf[:, m, h, :DN], ident[:])
                nc.vector.tensor_copy(aq0_sb[:, h, m, :], pt0[:])
                pt1 = ps_tr.tile([64, 128], BF, tag="tr", name="pt1_tr")
                nc.tensor.transpose(pt1[:], q_bf[:, m, h, DN:], ident[:])
                nc.vector.tensor_copy(aq1_sb[:, h, m, :], pt1[:])
            # stage this head's slab while later heads transpose
            eng = nc.gpsimd if h % 2 == 0 else nc.scalar
            eng.dma_start(out=_aqv[:DN, h], in_=aq0_sb[:, h])
            eng.dma_start(out=_aqv[DN:, h], in_=aq1_sb[:, h])
        nc.gpsimd.collective_compute(
            "AllToAll", mybir.AluOpType.bypass, replica_groups=RG,
            ins=[a2aq_in.opt()], outs=[a2aq_out.opt()])

        tmp_cm.__exit__(None, None, None)
        early_cm.__exit__(None, None, None)
        ps_x_cm.__exit__(None, None, None)

        # ---------------- Stage 4: gathered K/V setup ---------------------
        attn_cm = tc.tile_pool(name="attn", bufs=1)
        attn = attn_cm.__enter__()

        kT_sb = attn.tile([128, LC, T], BF, name="kT_sb")
        for lc in range(LC):
            nc.scalar.dma_start(
                out=kT_sb[:, lc].rearrange("p (s t) -> p s t", s=NCORES),
                in_=ag_out[:, lc * 128:(lc + 1) * 128, :].rearrange("s p t -> p s t"))
        kTpe_sb = attn.tile([64, T], BF, name="kTpe_sb")
        nc.scalar.dma_start(
            out=kTpe_sb[:].rearrange("p (s t) -> p s t", s=NCORES),
            in_=ag_out[:, KVL:, :].rearrange("s p t -> p s t"))

        v_sb = attn.tile([128, NQB, HLOC, DVE_], BF, name="v_sb")
        nc.vector.memset(v_sb[:, :, :, DV:], 1.0)
        for tcb in range(NQB):
            pv = ps_mm.tile([128, HLOC * DV], F32, tag="mm", name="pv_ps")
            for lc in range(LC):
                nc.tensor.matmul(pv[:], kT_sb[:, lc, tcb * 128:(tcb + 1) * 128],
                                 wvc_sb[:, lc, :], start=(lc == 0),
                                 stop=(lc == LC - 1))
            nc.vector.tensor_copy(
                v_sb[:, tcb, :, :DV],
                pv[:].rearrange("p (h v) -> p h v", h=HLOC))

        # ---------------- Stage 5: q^T for my heads -----------------------
        qT_sb = attn.tile([128, HLOC, T], BF, name="qT_sb")
        qTpe_sb = attn.tile([64, NQB, HLOC, 128], BF, name="qTpe_sb")
        for h in range(HLOC):
            nc.gpsimd.dma_start(
                out=qT_sb[:, h].rearrange("p (s t) -> p s t", s=NCORES),
                in_=a2aq_out[:, h, :DN, :].rearrange("s d t -> d s t"))
            for q2 in range(2):
                nc.scalar.dma_start(
                    out=qTpe_sb[:].rearrange("p (s q2) hh t -> p q2 s hh t",
                                             q2=2)[:, q2, :, h, :],
                    in_=a2aq_out[:, h, DN:, q2 * 128:(q2 + 1) * 128]
                        .rearrange("s d t -> d s t"))

        qabsT_sb = attn.tile([128, LC, NQB, HLOC, 128], BF, name="qabsT_sb")
        for tq in range(T // 512):
            for lc in range(LC):
                for h in range(HLOC):
                    pqa = ps_mm.tile([128, 512], F32, tag="mm", name="pqa_ps")
                    nc.tensor.matmul(pqa[:], wkc_sb[:, h, lc * 128:(lc + 1) * 128],
                                     qT_sb[:, h, tq * 512:(tq + 1) * 512],
                                     start=True, stop=True)
                    nc.scalar.copy(
                        qabsT_sb[:, lc, tq * 4:(tq + 1) * 4, h, :],
                        pqa[:].rearrange("p (q t) -> p q t", q=4))

        # W_O weight prefetch: DMA engines are idle during attention.
        # The first ring of tiles is gated on qabsT (a tiny WAW dep) so the
        # stream starts at attention entry, not during phase 1 where it
        # would steal weight-stream bandwidth and slow the collectives.
        WO_BUFS = 6
        wo_tiles = []
        for w in range(2 * NHT):
            wt = wo_pool.tile([128, (H // 2) * 512], BF, tag="wo_t",
                              bufs=WO_BUFS, name="wo_t")
            if w < WO_BUFS:
                nc.scalar.copy(wt[0:1, 0:1], kT_sb[0:1, 0, 0:1])
            nc.sync.dma_start(out=wt[:], in_=wo[w])
            wo_tiles.append(wt)

        # ---------------- Stage 6: attention ------------------------------
        ps_o_cm = tc.tile_pool(name="ps_o", bufs=2, space="PSUM")
        ps_o = ps_o_cm.__enter__()
        pexp_cm = tc.tile_pool(name="pexp", bufs=3)
        pexp = pexp_cm.__enter__()
        onorm_cm = tc.tile_pool(name="onorm", bufs=3)
        onorm = onorm_cm.__enter__()
        ao_sb = attn.tile([128, HLOC, NQB, 128], BF, name="ao_sb")

        for qj in range(NQB // 2):
            qb0, qb1 = 2 * qj, 2 * qj + 1
            pos = [ps_o.tile([128, HLOC, DVE_], F32, tag="po", name="po0"),
                   ps_o.tile([128, HLOC, DVE_], F32, tag="po", name="po1")]
            for kb in range(qb1 + 1):
                if kb <= qb0:
                    # both query blocks of the pair attend to this key block
                    psc = ps_mm.tile([128, 2, HLOC, 128], F32, tag="mm",
                                     name="psc2")
                    for lc in range(LC):
                        nc.tensor.matmul(
                            psc[:], kT_sb[:, lc, kb * 128:(kb + 1) * 128],
                            qabsT_sb[:, lc, qb0:qb1 + 1, :, :],
                            start=(lc == 0), stop=False)
                    nc.tensor.matmul(
                        psc[:], kTpe_sb[:, kb * 128:(kb + 1) * 128],
                        qTpe_sb[:, qb0:qb1 + 1, :, :],
                        start=False, stop=True)
                    p_bf = pexp.tile([128, 2, HLOC, 128], BF, tag="p_bf",
                                     name="p_bf")
                    nc.scalar.activation(p_bf[:], psc[:],
                                         mybir.ActivationFunctionType.Exp,
                                         scale=float(SCALE))
                    if kb == qb0:
                        nc.vector.tensor_mul(p_bf[:, 0], p_bf[:, 0], cmask_sb[:])
                    for qi in range(2):
                        for h in range(HLOC):
                            nc.tensor.matmul(
                                pos[qi][:, h, :], p_bf[:, qi, h, :],
                                v_sb[:, kb, h, :],
                                start=(kb == 0 and h == 0),
                                stop=(kb == (qb0 if qi == 0 else qb1)))
                else:
                    # kb == qb1: only the odd block (its diagonal)
                    psc1 = ps_mm.tile([128, HLOC, 128], F32, tag="mm",
                                      name="psc1")
                    for lc in range(LC):
                        nc.tensor.matmul(
                            psc1[:], kT_sb[:, lc, kb * 128:(kb + 1) * 128],
                            qabsT_sb[:, lc, qb1, :, :],
                            start=(lc == 0), stop=False)
                    nc.tensor.matmul(
                        psc1[:], kTpe_sb[:, kb * 128:(kb + 1) * 128],
                        qTpe_sb[:, qb1, :, :],
                        start=False, stop=True)
                    p1 = pexp.tile([128, HLOC, 128], BF, tag="p_bf", name="p1")
                    nc.scalar.activation(p1[:], psc1[:],
                                         mybir.ActivationFunctionType.Exp,
                                         scale=float(SCALE))
                    nc.vector.tensor_mul(p1[:], p1[:], cmask_sb[:])
                    for h in range(HLOC):
                        nc.tensor.matmul(
                            pos[1][:, h, :], p1[:, h, :],
                            v_sb[:, kb, h, :],
                            start=False, stop=True)
            for qi, qb in ((0, qb0), (1, qb1)):
                po = pos[qi]
                for h in range(HLOC):
                    rh = onorm.tile([128, 1], F32, tag="rh", name="rh")
                    nc.vector.reciprocal(rh[:], po[:, h, DV:DVE_])
                    ob = onorm.tile([128, DV], BF, tag="ob", name="ob")
                    nc.vector.tensor_scalar_mul(ob[:], in0=po[:, h, :DV],
                                                scalar1=rh[:])
                    pot = ps_tr.tile([128, 128], BF, tag="tr", name="pot_tr")
                    nc.tensor.transpose(pot[:], ob[:], ident[:])
                    nc.scalar.copy(ao_sb[:, h, qb, :], pot[:])
            # stage a2a chunk qj (dest core qj's tokens) while later pairs run
            for hh in range(HLOC):
                nc.gpsimd.dma_start(
                    out=a2ao_in[qj, hh, :, :].rearrange("v (q2 t) -> v q2 t", q2=2),
                    in_=ao_sb[:, hh, qb0:qb1 + 1, :])

        nc.gpsimd.collective_compute(
            "AllToAll", mybir.AluOpType.bypass, replica_groups=RG,
            ins=[a2ao_in.opt()], outs=[a2ao_out.opt()])

        # ---------------- Stage 7: W_O ------------------------------------
        oT_sb = attn.tile([128, H, TLOC], BF, name="oT_sb")
        _oTv = oT_sb[:].rearrange("p (s hh) t -> p s hh t", s=NCORES)
        _a2aov = a2ao_out[:, :, :, :].rearrange("s hh v t -> v s hh t")
        for _q, _eng in ((0, nc.gpsimd), (1, nc.scalar), (2, nc.gpsimd),
                         (3, nc.scalar)):
            nc_s = slice(_q * 2, _q * 2 + 2)
            _eng.dma_start(out=_oTv[:, nc_s], in_=_a2aov[:, nc_s])
        outp_cm = tc.tile_pool(name="outp", bufs=4)
        outp = outp_cm.__enter__()
        for ht in range(NHT):
            pso = [ps_mm.tile([128, 512], F32, tag="mm", name="wo_ps")
                   for _ in range(MCH)]
            for c in range(H):
                wt = wo_tiles[2 * ht + c // 8]
                for m in range(MCH):
                    nc.tensor.matmul(pso[m][:], oT_sb[:, c, m * 128:(m + 1) * 128],
                                     wt[:, (c % 8) * 512:(c % 8 + 1) * 512],
                                     start=(c == 0), stop=(c == H - 1))
            for m in range(MCH):
                ot = outp.tile([128, 512], F32, tag="ot", name="ot")
                if ht % 2 == 0:
                    nc.scalar.copy(ot[:], pso[m][:])
                else:
                    nc.vector.tensor_copy(ot[:], pso[m][:])
                nc.scalar.dma_start(
                    out=out[:, :].rearrange("(m p) d -> p m d", p=128)[
                        :, m, ht * 512:(ht + 1) * 512],
                    in_=ot[:])

        for p in (outp_cm, onorm_cm, pexp_cm, ps_o_cm, attn_cm, wo_cm,
                  ps_tr_cm, ps_mm_cm, wstream_cm, dram_cm, consts_cm):
            p.__exit__(None, None, None)

    nc.finalize()
    return nc


def _to_bf16(a):
    return np.asarray(a, dtype=np.float32).astype(ml_dtypes.bfloat16)


def _prep_in_maps(positions, hidden_states, w_fused, w_qb, w_kvb, w_o,
                  qa_ln_w, kva_ln_w):
    positions = np.asarray(positions)
    hidden_states = np.asarray(hidden_states, dtype=np.float32)
    w_fused = np.asarray(w_fused, dtype=np.float32)
    w_qb = np.asarray(w_qb, dtype=np.float32)
    w_kvb = np.asarray(w_kvb, dtype=np.float32)
    w_o = np.asarray(w_o, dtype=np.float32)
    qa_ln_w = np.asarray(qa_ln_w, dtype=np.float32)
    kva_ln_w = np.asarray(kva_ln_w, dtype=np.float32)

    inv_freq = 1.0 / (THETA ** (np.arange(0, DR, 2, dtype=np.float32) / DR))
    freqs = positions.astype(np.float32)[:, None] * inv_freq[None, :]
    cs_full = np.concatenate([np.cos(freqs), np.sin(freqs)], axis=1)  # [T, 64]

    wqb_folded = qa_ln_w[:, None] * w_qb
    wkvb_r = w_kvb.reshape(KVL, H, DN + DV)

    # wf kv+pe tiles: variable k-chunks per tile, rows contiguous per tile
    kvpe_cols = w_fused[:, QL:QL + KVPE]                       # [5120, 576]
    _blocks = []
    _k0 = 0
    for _nk in WF_KV_TILES:
        _blocks.append(
            kvpe_cols[_k0 * 128:(_k0 + _nk) * 128]
            .reshape(_nk, 128, KVPE).transpose(1, 0, 2).reshape(-1))
        _k0 += _nk
    wf_kv_arr = _to_bf16(np.concatenate(_blocks))
    # wf q tiles: [20][128][2*1536]
    q_cols = w_fused[:, :QL]                                   # [5120, 1536]
    wf_q_arr = _to_bf16(
        q_cols.reshape(NT_Q, 2, 128, QL).transpose(0, 2, 1, 3)
              .reshape(NT_Q, 128, 2 * QL))
    # wqb tiles: [half][k][128][1536]
    wqb_arr = _to_bf16(
        wqb_folded.reshape(QKD, 128, 2, QL).transpose(2, 0, 1, 3))
    wqb_arr = np.ascontiguousarray(wqb_arr)
    # wo tiles: [2*ht + c//8][p][(c%8)*512+j] = w_o[c*128+p, ht*512+j]
    wo_arr = _to_bf16(
        w_o.reshape(2, H // 2, 128, NHT, 512).transpose(3, 0, 2, 1, 4)
           .reshape(2 * NHT, 128, (H // 2) * 512))
    wo_arr = np.ascontiguousarray(wo_arr)

    tri = np.triu(np.ones((128, 128), np.float32))
    cmask = _to_bf16(np.repeat(tri[:, None, :], HLOC, axis=1))

    in_maps = []
    for c in range(NCORES):
        tok = slice(c * TLOC, (c + 1) * TLOC)
        heads = [HLOC * c + i for i in range(HLOC)]
        wkcT = np.stack([(wkvb_r[:, h, :DN] * kva_ln_w[:, None]).T for h in heads])
        wvc = np.concatenate(
            [wkvb_r[:, h, DN:] * kva_ln_w[:, None] for h in heads], axis=1)
        hT_arr = _to_bf16(np.ascontiguousarray(
            hidden_states[tok].T.reshape(KD, 128, TLOC).transpose(1, 0, 2)
                              .reshape(128, KD * TLOC)))
        in_maps.append({
            "hT": hT_arr,
            "wf_kv": wf_kv_arr,
            "wf_q": wf_q_arr,
            "wqb": wqb_arr,
            "cs": np.ascontiguousarray(cs_full[tok]),
            "wkcT": _to_bf16(np.ascontiguousarray(wkcT)),
            "wvc": _to_bf16(np.ascontiguousarray(wvc)),
            "wo": wo_arr,
            "cmask": cmask,
        })
    return in_maps


def kernel(**inputs):
    global _NC_CACHE, _last_in_maps
    in_maps = _prep_in_maps(**inputs)
    _last_in_maps = in_maps
    if _NC_CACHE is None:
        _NC_CACHE = build_nc()

    res = run_bass_kernel_spmd(_NC_CACHE, in_maps, core_ids=list(range(NCORES)))
    return np.concatenate([np.asarray(res.results[c]["out"], dtype=np.float32)
                           for c in range(NCORES)], axis=0)


if __name__ == "__main__":
    build_nc()
    print("build ok")


# revision 27
# speedup vs baseline: 1.0022x; 1.0022x over previous
"""DeepseekV2 MLA attention on 8 Trainium2 NeuronCores.

Sharding (uniform SPMD, no control divergence):
- A-projection, q-up-projection, final W_O: token-sharded (core c owns
  tokens [256c, 256c+256)).
- Attention (absorbed MLA over the compressed KV latent): head-sharded
  (core c owns heads {2c, 2c+1}).
- Three collectives connect the shardings: AllGather of the kv latent
  (feature-major, bf16), AllToAll of q^T (token->head resharding),
  AllToAll of normalized o^T (head->token resharding).

All matmuls run in bf16 with fp32 PSUM accumulation. RMSNorm weights are
folded into the adjacent weight matrices on the host. Softmax runs
unnormalized (logits are small by construction: std ~0.7) with the
denominator obtained by appending a ones-column to V; normalization is a
per-partition scale on the token-major attention output.

Weight matrices are repacked on the host into tiles whose SBUF-partition
rows are 3-16KB contiguous in DRAM so each DMA descriptor moves large
packets (the DMA ring round-robins packets over 16 engines; throughput
per engine is packet-size-bound). W_O tiles are prefetched during the
attention phase, when the DMA engines are otherwise idle.
"""

import os
import sys

for _p in ("/opt/trn_rl_repo", "/root/.axon_site", "/root/.axon_site/_ro/trn_rl_repo",
           "/root/.axon_site/_ro/pypackages"):
    if os.path.isdir(_p) and _p not in sys.path:
        sys.path.insert(0, _p)

import numpy as np
import ml_dtypes

import concourse.bass as bass
import concourse.tile as tile
from concourse import bacc, mybir
from concourse.bass_utils import run_bass_kernel_spmd
from concourse.masks import make_identity

# Problem constants (hardcoded per contract)
T, HID, H = 2048, 5120, 16
DN, DR, DV = 128, 64, 128
QL, KVL = 1536, 512
EPS = 1e-6
THETA = 10000.0
SCALE = (DN + DR) ** -0.5

NCORES = 8
TLOC = T // NCORES          # 256 tokens per core
HLOC = H // NCORES          # 2 heads per core
MCH = TLOC // 128           # 2 token chunks of 128
KD = HID // 128             # 40 contraction chunks for A-proj
QKD = QL // 128             # 12 contraction chunks for q-up
LC = KVL // 128             # 4 latent chunks
NQB = T // 128              # 16 query/key blocks of 128
DQK = DN + DR               # 192
DVE_ = DV + 1               # 129: extra ones-column for softmax denominator
KVPE = KVL + DR             # 576

WF_KV_TILES = [1, 1, 2] + [4] * 9   # k-chunks per wf_kv tile (sum 40)
NT_Q = 20                   # wf q tiles (2 k-chunks each)
NHT = HID // 512            # 10 W_O output groups

BF = mybir.dt.bfloat16
F32 = mybir.dt.float32

_NC_CACHE = None
_last_in_maps = None


def _rope_pair(nc, pool, x_pairs, cos, sin, out_pairs, shape):
    """Interleaved rope: out1 = x1*cos - x2*sin ; out2 = x2*cos + x1*sin."""
    x1, x2 = x_pairs[:, 0], x_pairs[:, 1]
    o1, o2 = out_pairs[:, 0], out_pairs[:, 1]
    tm1 = pool.tile([128] + shape, F32, tag="rope_tm1", name="rope_tm1")
    tm2 = pool.tile([128] + shape, F32, tag="rope_tm2", name="rope_tm2")
    tm3 = pool.tile([128] + shape, F32, tag="rope_tm3", name="rope_tm3")
    nc.vector.tensor_mul(tm1[:], x1, cos)
    nc.vector.tensor_mul(tm2[:], x2, sin)
    nc.vector.tensor_mul(tm3[:], x1, sin)
    nc.vector.tensor_sub(o1, tm1[:], tm2[:])
    nc.vector.tensor_mul(tm1[:], x2, cos)
    nc.vector.tensor_add(o2, tm1[:], tm3[:])


def build_nc():
    nc = bacc.Bacc("TRN2", target_bir_lowering=False, debug=False,
                   num_devices=NCORES)

    # host-repacked inputs; rows (last axis) are DRAM-contiguous per tile
    hT = nc.dram_tensor("hT", [128, KD * TLOC], BF, kind="ExternalInput")
    wf_kv = nc.dram_tensor("wf_kv", [KD * 128 * KVPE], BF,
                           kind="ExternalInput")
    wf_q = nc.dram_tensor("wf_q", [NT_Q, 128, 2 * QL], BF, kind="ExternalInput")
    wqb = nc.dram_tensor("wqb", [2, QKD, 128, QL], BF, kind="ExternalInput")
    cs = nc.dram_tensor("cs", [TLOC, DR], F32, kind="ExternalInput")
    wkcT = nc.dram_tensor("wkcT", [HLOC, DN, KVL], BF, kind="ExternalInput")
    wvc = nc.dram_tensor("wvc", [KVL, HLOC * DV], BF, kind="ExternalInput")
    wo = nc.dram_tensor("wo", [2 * NHT, 128, (H // 2) * 512], BF,
                        kind="ExternalInput")
    cmask = nc.dram_tensor("cmask", [128, HLOC, 128], BF, kind="ExternalInput")
    out = nc.dram_tensor("out", [TLOC, HID], F32, kind="ExternalOutput")

    RG = [list(range(NCORES))]

    with tile.TileContext(nc) as tc:
        consts_cm = tc.tile_pool(name="consts", bufs=1)
        consts = consts_cm.__enter__()
        dram_cm = tc.tile_pool(name="dram", bufs=1, space="DRAM")
        dram = dram_cm.__enter__()
        wstream_cm = tc.tile_pool(name="wstream", bufs=4)
        wstream = wstream_cm.__enter__()
        ps_mm_cm = tc.tile_pool(name="ps_mm", bufs=4, space="PSUM")
        ps_mm = ps_mm_cm.__enter__()
        ps_tr_cm = tc.tile_pool(name="ps_tr", bufs=2, space="PSUM")
        ps_tr = ps_tr_cm.__enter__()
        # W_O prefetch pool: opened before `early` so its bytes are never
        # reused from phase-1 tiles (no false WAR delaying the prefetch).
        wo_cm = tc.tile_pool(name="wo_pool", bufs=3)
        wo_pool = wo_cm.__enter__()
        ps_x_cm = tc.tile_pool(name="ps_x", bufs=2, space="PSUM")
        ps_x = ps_x_cm.__enter__()

        ident = consts.tile([128, 128], BF, name="ident")
        make_identity(nc, ident[:])
        cmask_sb = consts.tile([128, HLOC, 128], BF, name="cmask_sb")
        wkc_sb = consts.tile([128, HLOC, KVL], BF, name="wkc_sb")
        eps_sb = consts.tile([128, 1], F32, name="eps_sb")
        nc.vector.memset(eps_sb[:], float(EPS))
        wvc_sb = consts.tile([128, LC, HLOC * DV], BF, name="wvc_sb")

        # collective DRAM tiles
        ag_in = dram.tile([KVPE, TLOC], BF, name="ag_in")
        ag_out = dram.tile([NCORES, KVPE, TLOC], BF, addr_space="Shared",
                           name="ag_out")
        a2aq_in = dram.tile([NCORES, HLOC, DQK, TLOC], BF, name="a2aq_in")
        a2aq_out = dram.tile([NCORES, HLOC, DQK, TLOC], BF, name="a2aq_out")
        a2ao_in = dram.tile([NCORES, HLOC, DV, TLOC], BF, name="a2ao_in")
        a2ao_out = dram.tile([NCORES, HLOC, DV, TLOC], BF, name="a2ao_out")

        # ---------------- Phase 1: token-sharded projections ---------------
        early_cm = tc.tile_pool(name="early", bufs=1)
        early = early_cm.__enter__()
        tmp_cm = tc.tile_pool(name="tmp", bufs=1)
        tmp = tmp_cm.__enter__()

        hT_sb = early.tile([128, KD * TLOC], BF, name="hT_sb")

        def _hT_load(k0, nk):
            nc.sync.dma_start(out=hT_sb[:, k0 * TLOC:(k0 + nk) * TLOC],
                              in_=hT[:, k0 * TLOC:(k0 + nk) * TLOC])

        # first A-proj matmuls gate on a small slice; big slices interleave
        # with the wf_kv tile stream (issued inside the loop below)
        _hT_load(0, 2)
        _hT_load(2, 6)
        cs_sb = early.tile([128, MCH, DR], F32, name="cs_sb")
        nc.sync.dma_start(out=cs_sb[:],
                          in_=cs[:, :].rearrange("(m p) d -> p m d", p=128))

        qkv_sb = early.tile([128, MCH, QL + KVPE], F32, name="qkv_sb")

        def hT_lhs(k, m):
            return hT_sb[:, k * TLOC + m * 128: k * TLOC + m * 128 + 128]

        # --- A-proj, kv+pe columns (one fused pass over k) ---
        pss_kv = [ps_mm.tile([128, KVL], F32, tag="mm", name="aproj_kv")
                  for _ in range(MCH)]
        pss_pe = [ps_x.tile([128, DR], F32, tag="x", name="aproj_pe")
                  for _ in range(MCH)]
        _off = 0
        _k0 = 0
        for nk in WF_KV_TILES:
            wt = wstream.tile([128, 4 * KVPE], BF, tag="wf_kv", bufs=3,
                              name="wf_kv_t")
            nc.sync.dma_start(
                out=wt[:, :nk * KVPE],
                in_=wf_kv[_off:_off + 128 * nk * KVPE]
                    .rearrange("(p w) -> p w", p=128))
            _off += 128 * nk * KVPE
            if _k0 == 4:
                _hT_load(8, 16)
            elif _k0 == 16:
                _hT_load(24, 16)
            for kk in range(nk):
                k = _k0 + kk
                for m in range(MCH):
                    nc.tensor.matmul(pss_kv[m][:], hT_lhs(k, m),
                                     wt[:, kk * KVPE: kk * KVPE + KVL],
                                     start=(k == 0), stop=(k == KD - 1))
                    nc.tensor.matmul(pss_pe[m][:], hT_lhs(k, m),
                                     wt[:, kk * KVPE + KVL: (kk + 1) * KVPE],
                                     start=(k == 0), stop=(k == KD - 1))
            _k0 += nk
        for m in range(MCH):
            nc.vector.tensor_copy(qkv_sb[:, m, QL:QL + KVL], pss_kv[m][:])
            nc.scalar.copy(qkv_sb[:, m, QL + KVL:], pss_pe[m][:])

        # --- kv latent rmsnorm + rope(k_pe) + AllGather -------------------
        kvlat_bf = early.tile([128, MCH, KVL], BF, name="kvlat_bf")
        kpe_bf = early.tile([128, MCH, DR], BF, name="kpe_bf")
        agin_sb = early.tile([128, LC, MCH, 128], BF, name="agin_sb")
        agpe_sb = early.tile([64, MCH, 128], BF, name="agpe_sb")

        for m in range(MCH):
            sq = tmp.tile([128, KVL], F32, tag="sq_kv", name="sq_kv")
            ssum = tmp.tile([128, 1], F32, tag="ssum_kv", name="ssum_kv")
            nc.scalar.activation(sq[:], qkv_sb[:, m, QL:QL + KVL],
                                 mybir.ActivationFunctionType.Square,
                                 accum_out=ssum[:])
            rstd = tmp.tile([128, 1], F32, tag="rstd_kv", name="rstd_kv")
            nc.scalar.activation(rstd[:], ssum[:],
                                 mybir.ActivationFunctionType.Sqrt,
                                 bias=eps_sb[:], scale=1.0 / KVL)
            rinv = tmp.tile([128, 1], F32, tag="rinv_kv", name="rinv_kv")
            nc.vector.reciprocal(rinv[:], rstd[:])
            nc.vector.tensor_scalar_mul(kvlat_bf[:, m], in0=qkv_sb[:, m, QL:QL + KVL],
                                        scalar1=rinv[:])
            kv_pairs = qkv_sb[:, m, QL + KVL:].rearrange("p (i two) -> p two i", two=2)
            out_pairs = kpe_bf[:, m].rearrange("p (i two) -> p two i", two=2)
            _rope_pair(nc, tmp, kv_pairs,
                       cs_sb[:, m, :DR // 2], cs_sb[:, m, DR // 2:],
                       out_pairs, [DR // 2])
            for lc in range(LC):
                pt = ps_tr.tile([128, 128], BF, tag="tr", name="pt_tr")
                nc.tensor.transpose(pt[:], kvlat_bf[:, m, lc * 128:(lc + 1) * 128],
                                    ident[:])
                nc.vector.tensor_copy(agin_sb[:, lc, m, :], pt[:])
            ptp = ps_tr.tile([64, 128], BF, tag="tr", name="ptp_tr")
            nc.tensor.transpose(ptp[:], kpe_bf[:, m], ident[:])
            nc.vector.tensor_copy(agpe_sb[:, m, :], ptp[:])

        nc.gpsimd.dma_start(
            out=ag_in[:KVL, :].rearrange("(c p) m -> p c m", p=128)
                              .rearrange("p c (m t) -> p c m t", m=MCH),
            in_=agin_sb[:])
        nc.gpsimd.dma_start(
            out=ag_in[KVL:, :].rearrange("p (m t) -> p m t", m=MCH),
            in_=agpe_sb[:])
        nc.gpsimd.collective_compute(
            "AllGather", mybir.AluOpType.bypass, replica_groups=RG,
            ins=[ag_in.opt()], outs=[ag_out.opt()])

        # attention-phase consts: small loads tucked behind the q A-proj
        nc.sync.dma_start(out=wvc_sb[:],
                          in_=wvc[:, :].rearrange("(c p) v -> p c v", p=128))
        nc.sync.dma_start(out=wkc_sb[:], in_=wkcT[:, :, :].rearrange("h d l -> d h l"))
        nc.sync.dma_start(out=cmask_sb[:], in_=cmask[:, :, :])

        # --- A-proj, q columns (one fused pass over k) --------------------
        pss_q = [[None] * 3 for _ in range(MCH)]
        for m in range(MCH):
            for g in range(3):
                if m == 1 and g == 2:
                    pss_q[m][g] = ps_x.tile([128, 512], F32, tag="x",
                                            name="aproj_qx")
                elif m == 0 and g == 2:
                    pss_q[m][g] = ps_x.tile([128, 512], F32, tag="x",
                                            name="aproj_qx")
                else:
                    pss_q[m][g] = ps_mm.tile([128, 512], F32, tag="mm",
                                             name="aproj_q")
        for t in range(NT_Q):
            wt = wstream.tile([128, 2 * QL], BF, tag="wf_q", bufs=3,
                              name="wf_q_t")
            nc.sync.dma_start(out=wt[:], in_=wf_q[t])
            for kk in range(2):
                k = 2 * t + kk
                for m in range(MCH):
                    for g in range(3):
                        nc.tensor.matmul(
                            pss_q[m][g][:], hT_lhs(k, m),
                            wt[:, kk * QL + g * 512: kk * QL + (g + 1) * 512],
                            start=(k == 0), stop=(k == KD - 1))
        for m in range(MCH):
            for g in range(3):
                if g % 2 == 0:
                    nc.scalar.copy(qkv_sb[:, m, g * 512:(g + 1) * 512],
                                   pss_q[m][g][:])
                else:
                    nc.vector.tensor_copy(qkv_sb[:, m, g * 512:(g + 1) * 512],
                                          pss_q[m][g][:])

        # --- q rmsnorm + transpose --------------------------------------
        qan_bf = early.tile([128, MCH, QL], BF, name="qan_bf")
        for m in range(MCH):
            sq2 = tmp.tile([128, QL], F32, tag="sq_q", name="sq_q")
            ssum2 = tmp.tile([128, 1], F32, tag="ssum_q", name="ssum_q")
            nc.scalar.activation(sq2[:], qkv_sb[:, m, :QL],
                                 mybir.ActivationFunctionType.Square,
                                 accum_out=ssum2[:])
            rstd2 = tmp.tile([128, 1], F32, tag="rstd_q", name="rstd_q")
            nc.scalar.activation(rstd2[:], ssum2[:],
                                 mybir.ActivationFunctionType.Sqrt,
                                 bias=eps_sb[:], scale=1.0 / QL)
            rinv2 = tmp.tile([128, 1], F32, tag="rinv_q", name="rinv_q")
            nc.vector.reciprocal(rinv2[:], rstd2[:])
            nc.vector.tensor_scalar_mul(qan_bf[:, m], in0=qkv_sb[:, m, :QL],
                                        scalar1=rinv2[:])

        qanT_sb = early.tile([128, QKD, TLOC], BF, name="qanT_sb")
        for kc in range(QKD):
            for m in range(MCH):
                pt = ps_tr.tile([128, 128], BF, tag="tr", name="pt_tr")
                nc.tensor.transpose(pt[:], qan_bf[:, m, kc * 128:(kc + 1) * 128],
                                    ident[:])
                nc.vector.tensor_copy(qanT_sb[:, kc, m * 128:(m + 1) * 128], pt[:])

        # --- q-up projection: two column halves over cached qanT ----------
        q_bf = early.tile([128, MCH, H, DQK], BF, name="q_bf")
        for half in range(2):
            psq = [[None] * 3 for _ in range(MCH)]
            for m in range(MCH):
                for g in range(3):
                    if g == 2:
                        psq[m][g] = ps_x.tile([128, 512], F32, tag="x",
                                              name="qup_psx")
                    else:
                        psq[m][g] = ps_mm.tile([128, 512], F32, tag="mm",
                                               name="qup_ps")
            for kc in range(QKD):
                wqb_t = wstream.tile([128, QL], BF, tag="wqb_t", bufs=4,
                                     name="wqb_t")
                nc.sync.dma_start(out=wqb_t[:], in_=wqb[half, kc])
                for m in range(MCH):
                    for g in range(3):
                        nc.tensor.matmul(
                            psq[m][g][:], qanT_sb[:, kc, m * 128:(m + 1) * 128],
                            wqb_t[:, g * 512:(g + 1) * 512],
                            start=(kc == 0), stop=(kc == QKD - 1))
            for m in range(MCH):
                dst = q_bf[:, m].rearrange("p h d -> p (h d)")
                for g in range(3):
                    cols = slice(half * QL + g * 512, half * QL + (g + 1) * 512)
                    if g % 2 == 0:
                        nc.scalar.copy(dst[:, cols], psq[m][g][:])
                    else:
                        nc.vector.tensor_copy(dst[:, cols], psq[m][g][:])

        # --- rope on q_pe (in place, bf16) --------------------------------
        crep = early.tile([128, MCH, H, DR], F32, name="crep")
        _cs_base = cs[:, :]
        for m in range(MCH):
            nc.sync.dma_start(
                out=crep[:, m],
                in_=bass.AP(tensor=_cs_base.tensor, offset=m * 128 * DR,
                            ap=[[DR, 128], [0, H], [1, DR]]))
        for m in range(MCH):
            q_pairs = q_bf[:, m, :, DN:].rearrange("p h (i two) -> p two h i", two=2)
            _rope_pair(nc, tmp, q_pairs,
                       crep[:, m, :, :DR // 2], crep[:, m, :, DR // 2:],
                       q_pairs, [H, DR // 2])

        # --- transpose q and AllToAll (token -> head resharding) ----------
        aq0_sb = early.tile([128, H, MCH, 128], BF, name="aq0_sb")
        aq1_sb = early.tile([64, H, MCH, 128], BF, name="aq1_sb")
        _aqv = a2aq_in[:, :, :, :].rearrange("s hh d t -> (s hh) d t") \
                                  .rearrange("h d (m t) -> d h m t", m=MCH)
        for h in range(H):
            for m in range(MCH):
                pt0 = ps_tr.tile([128, 128], BF, tag="tr", name="pt0_tr")
                nc.tensor.transpose(pt0[:], q_bf[:, m, h, :DN], ident[:])
                nc.vector.tensor_copy(aq0_sb[:, h, m, :], pt0[:])
                pt1 = ps_tr.tile([64, 128], BF, tag="tr", name="pt1_tr")
                nc.tensor.transpose(pt1[:], q_bf[:, m, h, DN:], ident[:])
                nc.vector.tensor_copy(aq1_sb[:, h, m, :], pt1[:])
            # stage this head's slab while later heads transpose
            eng = nc.gpsimd if h % 2 == 0 else nc.scalar
            eng.dma_start(out=_aqv[:DN, h], in_=aq0_sb[:, h])
            eng.dma_start(out=_aqv[DN:, h], in_=aq1_sb[:, h])
        nc.gpsimd.collective_compute(
            "AllToAll", mybir.AluOpType.bypass, replica_groups=RG,
            ins=[a2aq_in.opt()], outs=[a2aq_out.opt()])

        tmp_cm.__exit__(None, None, None)
        early_cm.__exit__(None, None, None)
        ps_x_cm.__exit__(None, None, None)

        # ---------------- Stage 4: gathered K/V setup ---------------------
        attn_cm = tc.tile_pool(name="attn", bufs=1)
        attn = attn_cm.__enter__()

        kT_sb = attn.tile([128, LC, T], BF, name="kT_sb")
        for lc in range(LC):
            nc.scalar.dma_start(
                out=kT_sb[:, lc].rearrange("p (s t) -> p s t", s=NCORES),
                in_=ag_out[:, lc * 128:(lc + 1) * 128, :].rearrange("s p t -> p s t"))
        kTpe_sb = attn.tile([64, T], BF, name="kTpe_sb")
        nc.scalar.dma_start(
            out=kTpe_sb[:].rearrange("p (s t) -> p s t", s=NCORES),
            in_=ag_out[:, KVL:, :].rearrange("s p t -> p s t"))

        v_sb = attn.tile([128, NQB, HLOC, DVE_], BF, name="v_sb")
        nc.vector.memset(v_sb[:, :, :, DV:], 1.0)
        for tcb in range(NQB):
            pv = ps_mm.tile([128, HLOC * DV], F32, tag="mm", name="pv_ps")
            for lc in range(LC):
                nc.tensor.matmul(pv[:], kT_sb[:, lc, tcb * 128:(tcb + 1) * 128],
                                 wvc_sb[:, lc, :], start=(lc == 0),
                                 stop=(lc == LC - 1))
            nc.vector.tensor_copy(
                v_sb[:, tcb, :, :DV],
                pv[:].rearrange("p (h v) -> p h v", h=HLOC))

        # ---------------- Stage 5: q^T for my heads -----------------------
        qT_sb = attn.tile([128, HLOC, T], BF, name="qT_sb")
        qTpe_sb = attn.tile([64, NQB, HLOC, 128], BF, name="qTpe_sb")
        for h in range(HLOC):
            nc.gpsimd.dma_start(
                out=qT_sb[:, h].rearrange("p (s t) -> p s t", s=NCORES),
                in_=a2aq_out[:, h, :DN, :].rearrange("s d t -> d s t"))
            for q2 in range(2):
                nc.scalar.dma_start(
                    out=qTpe_sb[:].rearrange("p (s q2) hh t -> p q2 s hh t",
                                             q2=2)[:, q2, :, h, :],
                    in_=a2aq_out[:, h, DN:, q2 * 128:(q2 + 1) * 128]
                        .rearrange("s d t -> d s t"))

        qabsT_sb = attn.tile([128, LC, NQB, HLOC, 128], BF, name="qabsT_sb")
        for tq in range(T // 512):
            for lc in range(LC):
                for h in range(HLOC):
                    pqa = ps_mm.tile([128, 512], F32, tag="mm", name="pqa_ps")
                    nc.tensor.matmul(pqa[:], wkc_sb[:, h, lc * 128:(lc + 1) * 128],
                                     qT_sb[:, h, tq * 512:(tq + 1) * 512],
                                     start=True, stop=True)
                    nc.scalar.copy(
                        qabsT_sb[:, lc, tq * 4:(tq + 1) * 4, h, :],
                        pqa[:].rearrange("p (q t) -> p q t", q=4))

        # W_O weight prefetch: DMA engines are idle during attention.
        # The first ring of tiles is gated on qabsT (a tiny WAW dep) so the
        # stream starts at attention entry, not during phase 1 where it
        # would steal weight-stream bandwidth and slow the collectives.
        WO_BUFS = 6
        wo_tiles = []
        for w in range(2 * NHT):
            wt = wo_pool.tile([128, (H // 2) * 512], BF, tag="wo_t",
                              bufs=WO_BUFS, name="wo_t")
            if w < WO_BUFS:
                nc.scalar.copy(wt[0:1, 0:1], qT_sb[0:1, 0, 0:1])
            nc.sync.dma_start(out=wt[:], in_=wo[w])
            wo_tiles.append(wt)

        # ---------------- Stage 6: attention ------------------------------
        ps_o_cm = tc.tile_pool(name="ps_o", bufs=2, space="PSUM")
        ps_o = ps_o_cm.__enter__()
        pexp_cm = tc.tile_pool(name="pexp", bufs=3)
        pexp = pexp_cm.__enter__()
        onorm_cm = tc.tile_pool(name="onorm", bufs=3)
        onorm = onorm_cm.__enter__()
        ao_sb = attn.tile([128, HLOC, NQB, 128], BF, name="ao_sb")

        for qj in range(NQB // 2):
            qb0, qb1 = 2 * qj, 2 * qj + 1
            pos = [ps_o.tile([128, HLOC, DVE_], F32, tag="po", name="po0"),
                   ps_o.tile([128, HLOC, DVE_], F32, tag="po", name="po1")]
            for kb in range(qb1 + 1):
                if kb <= qb0:
                    # both query blocks of the pair attend to this key block
                    psc = ps_mm.tile([128, 2, HLOC, 128], F32, tag="mm",
                                     name="psc2")
                    for lc in range(LC):
                        nc.tensor.matmul(
                            psc[:], kT_sb[:, lc, kb * 128:(kb + 1) * 128],
                            qabsT_sb[:, lc, qb0:qb1 + 1, :, :],
                            start=(lc == 0), stop=False)
                    nc.tensor.matmul(
                        psc[:], kTpe_sb[:, kb * 128:(kb + 1) * 128],
                        qTpe_sb[:, qb0:qb1 + 1, :, :],
                        start=False, stop=True)
                    p_bf = pexp.tile([128, 2, HLOC, 128], BF, tag="p_bf",
                                     name="p_bf")
                    nc.scalar.activation(p_bf[:], psc[:],
                                         mybir.ActivationFunctionType.Exp,
                                         scale=float(SCALE))
                    if kb == qb0:
                        nc.vector.tensor_mul(p_bf[:, 0], p_bf[:, 0], cmask_sb[:])
                    for qi in range(2):
                        for h in range(HLOC):
                            nc.tensor.matmul(
                                pos[qi][:, h, :], p_bf[:, qi, h, :],
                                v_sb[:, kb, h, :],
                                start=(kb == 0 and h == 0),
                                stop=(kb == (qb0 if qi == 0 else qb1)))
                else:
                    # kb == qb1: only the odd block (its diagonal)
                    psc1 = ps_mm.tile([128, HLOC, 128], F32, tag="mm",
                                      name="psc1")
                    for lc in range(LC):
                        nc.tensor.matmul(
                            psc1[:], kT_sb[:, lc, kb * 128:(kb + 1) * 128],
                            qabsT_sb[:, lc, qb1, :, :],
                            start=(lc == 0), stop=False)
                    nc.tensor.matmul(
                        psc1[:], kTpe_sb[:, kb * 128:(kb + 1) * 128],
                        qTpe_sb[:, qb1, :, :],
                        start=False, stop=True)
                    p1 = pexp.tile([128, HLOC, 128], BF, tag="p_bf", name="p1")
                    nc.scalar.activation(p1[:], psc1[:],
                                         mybir.ActivationFunctionType.Exp,
                                         scale=float(SCALE))
                    nc.vector.tensor_mul(p1[:], p1[:], cmask_sb[:])
                    for h in range(HLOC):
                        nc.tensor.matmul(
                            pos[1][:, h, :], p1[:, h, :],
                            v_sb[:, kb, h, :],
                            start=False, stop=True)
            for qi, qb in ((0, qb0), (1, qb1)):
                po = pos[qi]
                for h in range(HLOC):
                    rh = onorm.tile([128, 1], F32, tag="rh", name="rh")
                    nc.vector.reciprocal(rh[:], po[:, h, DV:DVE_])
                    ob = onorm.tile([128, DV], BF, tag="ob", name="ob")
                    nc.vector.tensor_scalar_mul(ob[:], in0=po[:, h, :DV],
                                                scalar1=rh[:])
                    pot = ps_tr.tile([128, 128], BF, tag="tr", name="pot_tr")
                    nc.tensor.transpose(pot[:], ob[:], ident[:])
                    nc.scalar.copy(ao_sb[:, h, qb, :], pot[:])
            # stage a2a chunk qj (dest core qj's tokens) while later pairs run
            for hh in range(HLOC):
                nc.gpsimd.dma_start(
                    out=a2ao_in[qj, hh, :, :].rearrange("v (q2 t) -> v q2 t", q2=2),
                    in_=ao_sb[:, hh, qb0:qb1 + 1, :])

        nc.gpsimd.collective_compute(
            "AllToAll", mybir.AluOpType.bypass, replica_groups=RG,
            ins=[a2ao_in.opt()], outs=[a2ao_out.opt()])

        # ---------------- Stage 7: W_O ------------------------------------
        oT_sb = attn.tile([128, H, TLOC], BF, name="oT_sb")
        _oTv = oT_sb[:].rearrange("p (s hh) t -> p s hh t", s=NCORES)
        _a2aov = a2ao_out[:, :, :, :].rearrange("s hh v t -> v s hh t")
        for _q, _eng in ((0, nc.gpsimd), (1, nc.scalar), (2, nc.gpsimd),
                         (3, nc.scalar)):
            nc_s = slice(_q * 2, _q * 2 + 2)
            _eng.dma_start(out=_oTv[:, nc_s], in_=_a2aov[:, nc_s])
        outp_cm = tc.tile_pool(name="outp", bufs=4)
        outp = outp_cm.__enter__()
        for ht in range(NHT):
            pso = [ps_mm.tile([128, 512], F32, tag="mm", name="wo_ps")
                   for _ in range(MCH)]
            for c in range(H):
                wt = wo_tiles[2 * ht + c // 8]
                for m in range(MCH):
                    nc.tensor.matmul(pso[m][:], oT_sb[:, c, m * 128:(m + 1) * 128],
                                     wt[:, (c % 8) * 512:(c % 8 + 1) * 512],
                                     start=(c == 0), stop=(c == H - 1))
            for m in range(MCH):
                ot = outp.tile([128, 512], F32, tag="ot", name="ot")
                if ht % 2 == 0:
                    nc.scalar.copy(ot[:], pso[m][:])
                else:
                    nc.vector.tensor_copy(ot[:], pso[m][:])
                nc.scalar.dma_start(
                    out=out[:, :].rearrange("(m p) d -> p m d", p=128)[
                        :, m, ht * 512:(ht + 1) * 512],
                    in_=ot[:])

        for p in (outp_cm, onorm_cm, pexp_cm, ps_o_cm, attn_cm, wo_cm,
                  ps_tr_cm, ps_mm_cm, wstream_cm, dram_cm, consts_cm):
            p.__exit__(None, None, None)

    nc.finalize()
    return nc


def _to_bf16(a):
    return np.asarray(a, dtype=np.float32).astype(ml_dtypes.bfloat16)


def _prep_in_maps(positions, hidden_states, w_fused, w_qb, w_kvb, w_o,
                  qa_ln_w, kva_ln_w):
    positions = np.asarray(positions)
    hidden_states = np.asarray(hidden_states, dtype=np.float32)
    w_fused = np.asarray(w_fused, dtype=np.float32)
    w_qb = np.asarray(w_qb, dtype=np.float32)
    w_kvb = np.asarray(w_kvb, dtype=np.float32)
    w_o = np.asarray(w_o, dtype=np.float32)
    qa_ln_w = np.asarray(qa_ln_w, dtype=np.float32)
    kva_ln_w = np.asarray(kva_ln_w, dtype=np.float32)

    inv_freq = 1.0 / (THETA ** (np.arange(0, DR, 2, dtype=np.float32) / DR))
    freqs = positions.astype(np.float32)[:, None] * inv_freq[None, :]
    cs_full = np.concatenate([np.cos(freqs), np.sin(freqs)], axis=1)  # [T, 64]

    wqb_folded = qa_ln_w[:, None] * w_qb
    wkvb_r = w_kvb.reshape(KVL, H, DN + DV)

    # wf kv+pe tiles: variable k-chunks per tile, rows contiguous per tile
    kvpe_cols = w_fused[:, QL:QL + KVPE]                       # [5120, 576]
    _blocks = []
    _k0 = 0
    for _nk in WF_KV_TILES:
        _blocks.append(
            kvpe_cols[_k0 * 128:(_k0 + _nk) * 128]
            .reshape(_nk, 128, KVPE).transpose(1, 0, 2).reshape(-1))
        _k0 += _nk
    wf_kv_arr = _to_bf16(np.concatenate(_blocks))
    # wf q tiles: [20][128][2*1536]
    q_cols = w_fused[:, :QL]                                   # [5120, 1536]
    wf_q_arr = _to_bf16(
        q_cols.reshape(NT_Q, 2, 128, QL).transpose(0, 2, 1, 3)
              .reshape(NT_Q, 128, 2 * QL))
    # wqb tiles: [half][k][128][1536]
    wqb_arr = _to_bf16(
        wqb_folded.reshape(QKD, 128, 2, QL).transpose(2, 0, 1, 3))
    wqb_arr = np.ascontiguousarray(wqb_arr)
    # wo tiles: [2*ht + c//8][p][(c%8)*512+j] = w_o[c*128+p, ht*512+j]
    wo_arr = _to_bf16(
        w_o.reshape(2, H // 2, 128, NHT, 512).transpose(3, 0, 2, 1, 4)
           .reshape(2 * NHT, 128, (H // 2) * 512))
    wo_arr = np.ascontiguousarray(wo_arr)

    tri = np.triu(np.ones((128, 128), np.float32))
    cmask = _to_bf16(np.repeat(tri[:, None, :], HLOC, axis=1))

    in_maps = []
    for c in range(NCORES):
        tok = slice(c * TLOC, (c + 1) * TLOC)
        heads = [HLOC * c + i for i in range(HLOC)]
        wkcT = np.stack([(wkvb_r[:, h, :DN] * kva_ln_w[:, None]).T for h in heads])
        wvc = np.concatenate(
            [wkvb_r[:, h, DN:] * kva_ln_w[:, None] for h in heads], axis=1)
        hT_arr = _to_bf16(np.ascontiguousarray(
            hidden_states[tok].T.reshape(KD, 128, TLOC).transpose(1, 0, 2)
                              .reshape(128, KD * TLOC)))
        in_maps.append({
            "hT": hT_arr,
            "wf_kv": wf_kv_arr,
            "wf_q": wf_q_arr,
            "wqb": wqb_arr,
            "cs": np.ascontiguousarray(cs_full[tok]),
            "wkcT": _to_bf16(np.ascontiguousarray(wkcT)),
            "wvc": _to_bf16(np.ascontiguousarray(wvc)),
            "wo": wo_arr,
            "cmask": cmask,
        })
    return in_maps


def kernel(**inputs):
    global _NC_CACHE, _last_in_maps
    in_maps = _prep_in_maps(**inputs)
    _last_in_maps = in_maps
    if _NC_CACHE is None:
        _NC_CACHE = build_nc()

    res = run_bass_kernel_spmd(_NC_CACHE, in_maps, core_ids=list(range(NCORES)))
    return np.concatenate([np.asarray(res.results[c]["out"], dtype=np.float32)
                           for c in range(NCORES)], axis=0)


if __name__ == "__main__":
    build_nc()
    print("build ok")


# revision 28
# speedup vs baseline: 1.0146x; 1.0124x over previous
"""DeepseekV2 MLA attention on 8 Trainium2 NeuronCores.

Sharding (uniform SPMD, no control divergence):
- A-projection, q-up-projection, final W_O: token-sharded (core c owns
  tokens [256c, 256c+256)).
- Attention (absorbed MLA over the compressed KV latent): head-sharded
  (core c owns heads {2c, 2c+1}).
- Three collectives connect the shardings: AllGather of the kv latent
  (feature-major, bf16), AllToAll of q^T (token->head resharding),
  AllToAll of normalized o^T (head->token resharding).

All matmuls run in bf16 with fp32 PSUM accumulation. RMSNorm weights are
folded into the adjacent weight matrices on the host. Softmax runs
unnormalized (logits are small by construction: std ~0.7) with the
denominator obtained by appending a ones-column to V; normalization is a
per-partition scale on the token-major attention output.

Weight matrices are repacked on the host into tiles whose SBUF-partition
rows are 3-16KB contiguous in DRAM so each DMA descriptor moves large
packets (the DMA ring round-robins packets over 16 engines; throughput
per engine is packet-size-bound). W_O tiles are prefetched during the
attention phase, when the DMA engines are otherwise idle.
"""

import os
import sys

for _p in ("/opt/trn_rl_repo", "/root/.axon_site", "/root/.axon_site/_ro/trn_rl_repo",
           "/root/.axon_site/_ro/pypackages"):
    if os.path.isdir(_p) and _p not in sys.path:
        sys.path.insert(0, _p)

import numpy as np
import ml_dtypes

import concourse.bass as bass
import concourse.tile as tile
from concourse import bacc, mybir
from concourse.bass_utils import run_bass_kernel_spmd
from concourse.masks import make_identity

# Problem constants (hardcoded per contract)
T, HID, H = 2048, 5120, 16
DN, DR, DV = 128, 64, 128
QL, KVL = 1536, 512
EPS = 1e-6
THETA = 10000.0
SCALE = (DN + DR) ** -0.5

NCORES = 8
TLOC = T // NCORES          # 256 tokens per core
HLOC = H // NCORES          # 2 heads per core
MCH = TLOC // 128           # 2 token chunks of 128
KD = HID // 128             # 40 contraction chunks for A-proj
QKD = QL // 128             # 12 contraction chunks for q-up
LC = KVL // 128             # 4 latent chunks
NQB = T // 128              # 16 query/key blocks of 128
DQK = DN + DR               # 192
DVE_ = DV + 1               # 129: extra ones-column for softmax denominator
KVPE = KVL + DR             # 576

WF_KV_TILES = [1, 1, 2] + [4] * 9   # k-chunks per wf_kv tile (sum 40)
NT_Q = 20                   # wf q tiles (2 k-chunks each)
NHT = HID // 512            # 10 W_O output groups

BF = mybir.dt.bfloat16
F32 = mybir.dt.float32

_NC_CACHE = None
_last_in_maps = None


def _rope_pair(nc, pool, x_pairs, cos, sin, out_pairs, shape):
    """Interleaved rope: out1 = x1*cos - x2*sin ; out2 = x2*cos + x1*sin."""
    x1, x2 = x_pairs[:, 0], x_pairs[:, 1]
    o1, o2 = out_pairs[:, 0], out_pairs[:, 1]
    tm1 = pool.tile([128] + shape, F32, tag="rope_tm1", name="rope_tm1")
    tm2 = pool.tile([128] + shape, F32, tag="rope_tm2", name="rope_tm2")
    tm3 = pool.tile([128] + shape, F32, tag="rope_tm3", name="rope_tm3")
    nc.vector.tensor_mul(tm1[:], x1, cos)
    nc.vector.tensor_mul(tm2[:], x2, sin)
    nc.vector.tensor_mul(tm3[:], x1, sin)
    nc.vector.tensor_sub(o1, tm1[:], tm2[:])
    nc.vector.tensor_mul(tm1[:], x2, cos)
    nc.vector.tensor_add(o2, tm1[:], tm3[:])


def build_nc():
    nc = bacc.Bacc("TRN2", target_bir_lowering=False, debug=False,
                   num_devices=NCORES)

    # host-repacked inputs; rows (last axis) are DRAM-contiguous per tile
    hT = nc.dram_tensor("hT", [128, KD * TLOC], BF, kind="ExternalInput")
    wf_kv = nc.dram_tensor("wf_kv", [KD * 128 * KVPE], BF,
                           kind="ExternalInput")
    wf_q = nc.dram_tensor("wf_q", [NT_Q, 128, 2 * QL], BF, kind="ExternalInput")
    wqb = nc.dram_tensor("wqb", [2, QKD, 128, QL], BF, kind="ExternalInput")
    cs = nc.dram_tensor("cs", [TLOC, DR], F32, kind="ExternalInput")
    wkcT = nc.dram_tensor("wkcT", [HLOC, DN, KVL], BF, kind="ExternalInput")
    wvc = nc.dram_tensor("wvc", [KVL, HLOC * DV], BF, kind="ExternalInput")
    wo = nc.dram_tensor("wo", [2 * NHT, 128, (H // 2) * 512], BF,
                        kind="ExternalInput")
    cmask = nc.dram_tensor("cmask", [128, HLOC, 128], BF, kind="ExternalInput")
    out = nc.dram_tensor("out", [TLOC, HID], F32, kind="ExternalOutput")

    RG = [list(range(NCORES))]

    with tile.TileContext(nc) as tc:
        consts_cm = tc.tile_pool(name="consts", bufs=1)
        consts = consts_cm.__enter__()
        dram_cm = tc.tile_pool(name="dram", bufs=1, space="DRAM")
        dram = dram_cm.__enter__()
        wstream_cm = tc.tile_pool(name="wstream", bufs=4)
        wstream = wstream_cm.__enter__()
        ps_mm_cm = tc.tile_pool(name="ps_mm", bufs=4, space="PSUM")
        ps_mm = ps_mm_cm.__enter__()
        ps_tr_cm = tc.tile_pool(name="ps_tr", bufs=2, space="PSUM")
        ps_tr = ps_tr_cm.__enter__()
        # W_O prefetch pool: opened before `early` so its bytes are never
        # reused from phase-1 tiles (no false WAR delaying the prefetch).
        wo_cm = tc.tile_pool(name="wo_pool", bufs=3)
        wo_pool = wo_cm.__enter__()
        ps_x_cm = tc.tile_pool(name="ps_x", bufs=2, space="PSUM")
        ps_x = ps_x_cm.__enter__()

        ident = consts.tile([128, 128], BF, name="ident")
        make_identity(nc, ident[:])
        cmask_sb = consts.tile([128, HLOC, 128], BF, name="cmask_sb")
        wkc_sb = consts.tile([128, HLOC, KVL], BF, name="wkc_sb")
        eps_sb = consts.tile([128, 1], F32, name="eps_sb")
        nc.vector.memset(eps_sb[:], float(EPS))
        wvc_sb = consts.tile([128, LC, HLOC * DV], BF, name="wvc_sb")

        # collective DRAM tiles
        ag_in = dram.tile([KVPE, TLOC], BF, name="ag_in")
        ag_out = dram.tile([NCORES, KVPE, TLOC], BF, addr_space="Shared",
                           name="ag_out")
        a2aq_in = dram.tile([NCORES, HLOC, DQK, TLOC], BF, name="a2aq_in")
        a2aq_out = dram.tile([NCORES, HLOC, DQK, TLOC], BF, name="a2aq_out")
        a2ao_in = dram.tile([NCORES, HLOC, DV, TLOC], BF, name="a2ao_in")
        a2ao_out = dram.tile([NCORES, HLOC, DV, TLOC], BF, name="a2ao_out")

        # ---------------- Phase 1: token-sharded projections ---------------
        early_cm = tc.tile_pool(name="early", bufs=1)
        early = early_cm.__enter__()
        tmp_cm = tc.tile_pool(name="tmp", bufs=1)
        tmp = tmp_cm.__enter__()

        hT_sb = early.tile([128, KD * TLOC], BF, name="hT_sb")

        def _hT_load(k0, nk):
            nc.sync.dma_start(out=hT_sb[:, k0 * TLOC:(k0 + nk) * TLOC],
                              in_=hT[:, k0 * TLOC:(k0 + nk) * TLOC])

        # first A-proj matmuls gate on a small slice; big slices interleave
        # with the wf_kv tile stream (issued inside the loop below)
        _hT_load(0, 2)
        _hT_load(2, 6)
        cs_sb = early.tile([128, MCH, DR], F32, name="cs_sb")
        nc.sync.dma_start(out=cs_sb[:],
                          in_=cs[:, :].rearrange("(m p) d -> p m d", p=128))

        qkv_sb = early.tile([128, MCH, QL + KVPE], F32, name="qkv_sb")

        def hT_lhs(k, m):
            return hT_sb[:, k * TLOC + m * 128: k * TLOC + m * 128 + 128]

        # --- A-proj, kv+pe columns (one fused pass over k) ---
        pss_kv = [ps_mm.tile([128, KVL], F32, tag="mm", name="aproj_kv")
                  for _ in range(MCH)]
        pss_pe = [ps_x.tile([128, DR], F32, tag="x", name="aproj_pe")
                  for _ in range(MCH)]
        _off = 0
        _k0 = 0
        for nk in WF_KV_TILES:
            wt = wstream.tile([128, 4 * KVPE], BF, tag="wf_kv", bufs=3,
                              name="wf_kv_t")
            nc.sync.dma_start(
                out=wt[:, :nk * KVPE],
                in_=wf_kv[_off:_off + 128 * nk * KVPE]
                    .rearrange("(p w) -> p w", p=128))
            _off += 128 * nk * KVPE
            if _k0 == 4:
                _hT_load(8, 16)
            elif _k0 == 16:
                _hT_load(24, 16)
            for kk in range(nk):
                k = _k0 + kk
                for m in range(MCH):
                    nc.tensor.matmul(pss_kv[m][:], hT_lhs(k, m),
                                     wt[:, kk * KVPE: kk * KVPE + KVL],
                                     start=(k == 0), stop=(k == KD - 1))
                    nc.tensor.matmul(pss_pe[m][:], hT_lhs(k, m),
                                     wt[:, kk * KVPE + KVL: (kk + 1) * KVPE],
                                     start=(k == 0), stop=(k == KD - 1))
            _k0 += nk
        for m in range(MCH):
            nc.vector.tensor_copy(qkv_sb[:, m, QL:QL + KVL], pss_kv[m][:])
            nc.scalar.copy(qkv_sb[:, m, QL + KVL:], pss_pe[m][:])

        # --- kv latent rmsnorm + rope(k_pe) + AllGather -------------------
        kvlat_bf = early.tile([128, MCH, KVL], BF, name="kvlat_bf")
        kpe_bf = early.tile([128, MCH, DR], BF, name="kpe_bf")
        agin_sb = early.tile([128, LC, MCH, 128], BF, name="agin_sb")
        agpe_sb = early.tile([64, MCH, 128], BF, name="agpe_sb")

        for m in range(MCH):
            sq = tmp.tile([128, KVL], F32, tag="sq_kv", name="sq_kv")
            ssum = tmp.tile([128, 1], F32, tag="ssum_kv", name="ssum_kv")
            nc.scalar.activation(sq[:], qkv_sb[:, m, QL:QL + KVL],
                                 mybir.ActivationFunctionType.Square,
                                 accum_out=ssum[:])
            rstd = tmp.tile([128, 1], F32, tag="rstd_kv", name="rstd_kv")
            nc.scalar.activation(rstd[:], ssum[:],
                                 mybir.ActivationFunctionType.Sqrt,
                                 bias=eps_sb[:], scale=1.0 / KVL)
            rinv = tmp.tile([128, 1], F32, tag="rinv_kv", name="rinv_kv")
            nc.vector.reciprocal(rinv[:], rstd[:])
            nc.vector.tensor_scalar_mul(kvlat_bf[:, m], in0=qkv_sb[:, m, QL:QL + KVL],
                                        scalar1=rinv[:])
            kv_pairs = qkv_sb[:, m, QL + KVL:].rearrange("p (i two) -> p two i", two=2)
            out_pairs = kpe_bf[:, m].rearrange("p (i two) -> p two i", two=2)
            _rope_pair(nc, tmp, kv_pairs,
                       cs_sb[:, m, :DR // 2], cs_sb[:, m, DR // 2:],
                       out_pairs, [DR // 2])
            for lc in range(LC):
                pt = ps_tr.tile([128, 128], BF, tag="tr", name="pt_tr")
                nc.tensor.transpose(pt[:], kvlat_bf[:, m, lc * 128:(lc + 1) * 128],
                                    ident[:])
                nc.vector.tensor_copy(agin_sb[:, lc, m, :], pt[:])
            ptp = ps_tr.tile([64, 128], BF, tag="tr", name="ptp_tr")
            nc.tensor.transpose(ptp[:], kpe_bf[:, m], ident[:])
            nc.vector.tensor_copy(agpe_sb[:, m, :], ptp[:])

        nc.gpsimd.dma_start(
            out=ag_in[:KVL, :].rearrange("(c p) m -> p c m", p=128)
                              .rearrange("p c (m t) -> p c m t", m=MCH),
            in_=agin_sb[:])
        nc.gpsimd.dma_start(
            out=ag_in[KVL:, :].rearrange("p (m t) -> p m t", m=MCH),
            in_=agpe_sb[:])
        nc.gpsimd.collective_compute(
            "AllGather", mybir.AluOpType.bypass, replica_groups=RG,
            ins=[ag_in.opt()], outs=[ag_out.opt()])

        # attention-phase consts: small loads tucked behind the q A-proj
        nc.sync.dma_start(out=wvc_sb[:],
                          in_=wvc[:, :].rearrange("(c p) v -> p c v", p=128))
        nc.sync.dma_start(out=wkc_sb[:], in_=wkcT[:, :, :].rearrange("h d l -> d h l"))
        nc.sync.dma_start(out=cmask_sb[:], in_=cmask[:, :, :])

        # --- A-proj, q columns (one fused pass over k) --------------------
        pss_q = [[None] * 3 for _ in range(MCH)]
        for m in range(MCH):
            for g in range(3):
                if m == 1 and g == 2:
                    pss_q[m][g] = ps_x.tile([128, 512], F32, tag="x",
                                            name="aproj_qx")
                elif m == 0 and g == 2:
                    pss_q[m][g] = ps_x.tile([128, 512], F32, tag="x",
                                            name="aproj_qx")
                else:
                    pss_q[m][g] = ps_mm.tile([128, 512], F32, tag="mm",
                                             name="aproj_q")
        for t in range(NT_Q):
            wt = wstream.tile([128, 2 * QL], BF, tag="wf_q", bufs=3,
                              name="wf_q_t")
            nc.sync.dma_start(out=wt[:], in_=wf_q[t])
            for kk in range(2):
                k = 2 * t + kk
                for m in range(MCH):
                    for g in range(3):
                        nc.tensor.matmul(
                            pss_q[m][g][:], hT_lhs(k, m),
                            wt[:, kk * QL + g * 512: kk * QL + (g + 1) * 512],
                            start=(k == 0), stop=(k == KD - 1))
        for m in range(MCH):
            for g in range(3):
                if g % 2 == 0:
                    nc.scalar.copy(qkv_sb[:, m, g * 512:(g + 1) * 512],
                                   pss_q[m][g][:])
                else:
                    nc.vector.tensor_copy(qkv_sb[:, m, g * 512:(g + 1) * 512],
                                          pss_q[m][g][:])

        # --- q rmsnorm + transpose --------------------------------------
        qan_bf = early.tile([128, MCH, QL], BF, name="qan_bf")
        for m in range(MCH):
            sq2 = tmp.tile([128, QL], F32, tag="sq_q", name="sq_q")
            ssum2 = tmp.tile([128, 1], F32, tag="ssum_q", name="ssum_q")
            nc.scalar.activation(sq2[:], qkv_sb[:, m, :QL],
                                 mybir.ActivationFunctionType.Square,
                                 accum_out=ssum2[:])
            rstd2 = tmp.tile([128, 1], F32, tag="rstd_q", name="rstd_q")
            nc.scalar.activation(rstd2[:], ssum2[:],
                                 mybir.ActivationFunctionType.Sqrt,
                                 bias=eps_sb[:], scale=1.0 / QL)
            rinv2 = tmp.tile([128, 1], F32, tag="rinv_q", name="rinv_q")
            nc.vector.reciprocal(rinv2[:], rstd2[:])
            nc.vector.tensor_scalar_mul(qan_bf[:, m], in0=qkv_sb[:, m, :QL],
                                        scalar1=rinv2[:])

        qanT_sb = early.tile([128, QKD, TLOC], BF, name="qanT_sb")
        for kc in range(QKD):
            for m in range(MCH):
                pt = ps_tr.tile([128, 128], BF, tag="tr", name="pt_tr")
                nc.tensor.transpose(pt[:], qan_bf[:, m, kc * 128:(kc + 1) * 128],
                                    ident[:])
                nc.vector.tensor_copy(qanT_sb[:, kc, m * 128:(m + 1) * 128], pt[:])

        # --- q-up projection: two column halves over cached qanT ----------
        q_bf = early.tile([128, MCH, H, DQK], BF, name="q_bf")
        for half in range(2):
            psq = [[None] * 3 for _ in range(MCH)]
            for m in range(MCH):
                for g in range(3):
                    if g == 2:
                        psq[m][g] = ps_x.tile([128, 512], F32, tag="x",
                                              name="qup_psx")
                    else:
                        psq[m][g] = ps_mm.tile([128, 512], F32, tag="mm",
                                               name="qup_ps")
            for kc in range(QKD):
                wqb_t = wstream.tile([128, QL], BF, tag="wqb_t", bufs=4,
                                     name="wqb_t")
                nc.sync.dma_start(out=wqb_t[:], in_=wqb[half, kc])
                for m in range(MCH):
                    for g in range(3):
                        nc.tensor.matmul(
                            psq[m][g][:], qanT_sb[:, kc, m * 128:(m + 1) * 128],
                            wqb_t[:, g * 512:(g + 1) * 512],
                            start=(kc == 0), stop=(kc == QKD - 1))
            for m in range(MCH):
                dst = q_bf[:, m].rearrange("p h d -> p (h d)")
                for g in range(3):
                    cols = slice(half * QL + g * 512, half * QL + (g + 1) * 512)
                    if g % 2 == 0:
                        nc.scalar.copy(dst[:, cols], psq[m][g][:])
                    else:
                        nc.vector.tensor_copy(dst[:, cols], psq[m][g][:])

        # --- rope on q_pe (in place, bf16) --------------------------------
        crep = early.tile([128, MCH, H, DR], F32, name="crep")
        _cs_base = cs[:, :]
        for m in range(MCH):
            nc.sync.dma_start(
                out=crep[:, m],
                in_=bass.AP(tensor=_cs_base.tensor, offset=m * 128 * DR,
                            ap=[[DR, 128], [0, H], [1, DR]]))
        for m in range(MCH):
            q_pairs = q_bf[:, m, :, DN:].rearrange("p h (i two) -> p two h i", two=2)
            _rope_pair(nc, tmp, q_pairs,
                       crep[:, m, :, :DR // 2], crep[:, m, :, DR // 2:],
                       q_pairs, [H, DR // 2])

        # --- transpose q and AllToAll (token -> head resharding) ----------
        aq0_sb = early.tile([128, H, MCH, 128], BF, name="aq0_sb")
        aq1_sb = early.tile([64, H, MCH, 128], BF, name="aq1_sb")
        _aqv = a2aq_in[:, :, :, :].rearrange("s hh d t -> (s hh) d t") \
                                  .rearrange("h d (m t) -> d h m t", m=MCH)
        for h in range(H):
            for m in range(MCH):
                pt0 = ps_tr.tile([128, 128], BF, tag="tr", name="pt0_tr")
                nc.tensor.transpose(pt0[:], q_bf[:, m, h, :DN], ident[:])
                nc.vector.tensor_copy(aq0_sb[:, h, m, :], pt0[:])
                pt1 = ps_tr.tile([64, 128], BF, tag="tr", name="pt1_tr")
                nc.tensor.transpose(pt1[:], q_bf[:, m, h, DN:], ident[:])
                nc.vector.tensor_copy(aq1_sb[:, h, m, :], pt1[:])
            # stage this head's slab while later heads transpose
            eng = nc.gpsimd if h % 2 == 0 else nc.scalar
            eng.dma_start(out=_aqv[:DN, h], in_=aq0_sb[:, h])
            eng.dma_start(out=_aqv[DN:, h], in_=aq1_sb[:, h])
        nc.gpsimd.collective_compute(
            "AllToAll", mybir.AluOpType.bypass, replica_groups=RG,
            ins=[a2aq_in.opt()], outs=[a2aq_out.opt()])

        tmp_cm.__exit__(None, None, None)
        early_cm.__exit__(None, None, None)
        ps_x_cm.__exit__(None, None, None)

        # ---------------- Stage 4: gathered K/V setup ---------------------
        attn_cm = tc.tile_pool(name="attn", bufs=1)
        attn = attn_cm.__enter__()

        kT_sb = attn.tile([128, LC, T], BF, name="kT_sb")
        for lc in range(LC):
            nc.scalar.dma_start(
                out=kT_sb[:, lc].rearrange("p (s t) -> p s t", s=NCORES),
                in_=ag_out[:, lc * 128:(lc + 1) * 128, :].rearrange("s p t -> p s t"))
        kTpe_sb = attn.tile([64, T], BF, name="kTpe_sb")
        nc.scalar.dma_start(
            out=kTpe_sb[:].rearrange("p (s t) -> p s t", s=NCORES),
            in_=ag_out[:, KVL:, :].rearrange("s p t -> p s t"))

        v_sb = attn.tile([128, NQB, HLOC, DVE_], BF, name="v_sb")
        nc.vector.memset(v_sb[:, :, :, DV:], 1.0)
        for tcb in range(NQB):
            pv = ps_mm.tile([128, HLOC * DV], F32, tag="mm", name="pv_ps")
            for lc in range(LC):
                nc.tensor.matmul(pv[:], kT_sb[:, lc, tcb * 128:(tcb + 1) * 128],
                                 wvc_sb[:, lc, :], start=(lc == 0),
                                 stop=(lc == LC - 1))
            nc.vector.tensor_copy(
                v_sb[:, tcb, :, :DV],
                pv[:].rearrange("p (h v) -> p h v", h=HLOC))

        # PE warm-keepers: the HAM clock throttle downshifts the PE to 4/8
        # duty after idle stretches; junk matmuls through the a2aq window
        # keep the activity up so attention starts at full clock.
        junk_ps = ps_mm.tile([128, 512], F32, tag="mm", name="junk_ps")
        for _f in range(40):
            nc.tensor.matmul(junk_ps[:], kT_sb[:, 0, :128], kT_sb[:, 1, :512],
                             start=True, stop=True)

        # ---------------- Stage 5: q^T for my heads -----------------------
        qT_sb = attn.tile([128, HLOC, T], BF, name="qT_sb")
        qTpe_sb = attn.tile([64, NQB, HLOC, 128], BF, name="qTpe_sb")
        for h in range(HLOC):
            nc.gpsimd.dma_start(
                out=qT_sb[:, h].rearrange("p (s t) -> p s t", s=NCORES),
                in_=a2aq_out[:, h, :DN, :].rearrange("s d t -> d s t"))
            for q2 in range(2):
                nc.scalar.dma_start(
                    out=qTpe_sb[:].rearrange("p (s q2) hh t -> p q2 s hh t",
                                             q2=2)[:, q2, :, h, :],
                    in_=a2aq_out[:, h, DN:, q2 * 128:(q2 + 1) * 128]
                        .rearrange("s d t -> d s t"))

        qabsT_sb = attn.tile([128, LC, NQB, HLOC, 128], BF, name="qabsT_sb")
        for tq in range(T // 512):
            for lc in range(LC):
                for h in range(HLOC):
                    pqa = ps_mm.tile([128, 512], F32, tag="mm", name="pqa_ps")
                    nc.tensor.matmul(pqa[:], wkc_sb[:, h, lc * 128:(lc + 1) * 128],
                                     qT_sb[:, h, tq * 512:(tq + 1) * 512],
                                     start=True, stop=True)
                    nc.scalar.copy(
                        qabsT_sb[:, lc, tq * 4:(tq + 1) * 4, h, :],
                        pqa[:].rearrange("p (q t) -> p q t", q=4))

        # W_O weight prefetch: DMA engines are idle during attention.
        # The first ring of tiles is gated on qabsT (a tiny WAW dep) so the
        # stream starts at attention entry, not during phase 1 where it
        # would steal weight-stream bandwidth and slow the collectives.
        WO_BUFS = 6
        wo_tiles = []
        for w in range(2 * NHT):
            wt = wo_pool.tile([128, (H // 2) * 512], BF, tag="wo_t",
                              bufs=WO_BUFS, name="wo_t")
            if w < WO_BUFS:
                nc.scalar.copy(wt[0:1, 0:1], qT_sb[0:1, 0, 0:1])
            nc.sync.dma_start(out=wt[:], in_=wo[w])
            wo_tiles.append(wt)

        # ---------------- Stage 6: attention ------------------------------
        ps_o_cm = tc.tile_pool(name="ps_o", bufs=2, space="PSUM")
        ps_o = ps_o_cm.__enter__()
        pexp_cm = tc.tile_pool(name="pexp", bufs=3)
        pexp = pexp_cm.__enter__()
        onorm_cm = tc.tile_pool(name="onorm", bufs=3)
        onorm = onorm_cm.__enter__()
        ao_sb = attn.tile([128, HLOC, NQB, 128], BF, name="ao_sb")

        for qj in range(NQB // 2):
            qb0, qb1 = 2 * qj, 2 * qj + 1
            pos = [ps_o.tile([128, HLOC, DVE_], F32, tag="po", name="po0"),
                   ps_o.tile([128, HLOC, DVE_], F32, tag="po", name="po1")]
            for kb in range(qb1 + 1):
                if kb <= qb0:
                    # both query blocks of the pair attend to this key block
                    psc = ps_mm.tile([128, 2, HLOC, 128], F32, tag="mm",
                                     name="psc2")
                    for lc in range(LC):
                        nc.tensor.matmul(
                            psc[:], kT_sb[:, lc, kb * 128:(kb + 1) * 128],
                            qabsT_sb[:, lc, qb0:qb1 + 1, :, :],
                            start=(lc == 0), stop=False)
                    nc.tensor.matmul(
                        psc[:], kTpe_sb[:, kb * 128:(kb + 1) * 128],
                        qTpe_sb[:, qb0:qb1 + 1, :, :],
                        start=False, stop=True)
                    p_bf = pexp.tile([128, 2, HLOC, 128], BF, tag="p_bf",
                                     name="p_bf")
                    nc.scalar.activation(p_bf[:], psc[:],
                                         mybir.ActivationFunctionType.Exp,
                                         scale=float(SCALE))
                    if kb == qb0:
                        nc.vector.tensor_mul(p_bf[:, 0], p_bf[:, 0], cmask_sb[:])
                    for qi in range(2):
                        for h in range(HLOC):
                            nc.tensor.matmul(
                                pos[qi][:, h, :], p_bf[:, qi, h, :],
                                v_sb[:, kb, h, :],
                                start=(kb == 0 and h == 0),
                                stop=(kb == (qb0 if qi == 0 else qb1)))
                else:
                    # kb == qb1: only the odd block (its diagonal)
                    psc1 = ps_mm.tile([128, HLOC, 128], F32, tag="mm",
                                      name="psc1")
                    for lc in range(LC):
                        nc.tensor.matmul(
                            psc1[:], kT_sb[:, lc, kb * 128:(kb + 1) * 128],
                            qabsT_sb[:, lc, qb1, :, :],
                            start=(lc == 0), stop=False)
                    nc.tensor.matmul(
                        psc1[:], kTpe_sb[:, kb * 128:(kb + 1) * 128],
                        qTpe_sb[:, qb1, :, :],
                        start=False, stop=True)
                    p1 = pexp.tile([128, HLOC, 128], BF, tag="p_bf", name="p1")
                    nc.scalar.activation(p1[:], psc1[:],
                                         mybir.ActivationFunctionType.Exp,
                                         scale=float(SCALE))
                    nc.vector.tensor_mul(p1[:], p1[:], cmask_sb[:])
                    for h in range(HLOC):
                        nc.tensor.matmul(
                            pos[1][:, h, :], p1[:, h, :],
                            v_sb[:, kb, h, :],
                            start=False, stop=True)
            for qi, qb in ((0, qb0), (1, qb1)):
                po = pos[qi]
                for h in range(HLOC):
                    rh = onorm.tile([128, 1], F32, tag="rh", name="rh")
                    nc.vector.reciprocal(rh[:], po[:, h, DV:DVE_])
                    ob = onorm.tile([128, DV], BF, tag="ob", name="ob")
                    nc.vector.tensor_scalar_mul(ob[:], in0=po[:, h, :DV],
                                                scalar1=rh[:])
                    pot = ps_tr.tile([128, 128], BF, tag="tr", name="pot_tr")
                    nc.tensor.transpose(pot[:], ob[:], ident[:])
                    nc.scalar.copy(ao_sb[:, h, qb, :], pot[:])
            # stage a2a chunk qj (dest core qj's tokens) while later pairs run
            for hh in range(HLOC):
                nc.gpsimd.dma_start(
                    out=a2ao_in[qj, hh, :, :].rearrange("v (q2 t) -> v q2 t", q2=2),
                    in_=ao_sb[:, hh, qb0:qb1 + 1, :])

        nc.gpsimd.collective_compute(
            "AllToAll", mybir.AluOpType.bypass, replica_groups=RG,
            ins=[a2ao_in.opt()], outs=[a2ao_out.opt()])

        junk2_ps = ps_mm.tile([128, 512], F32, tag="mm", name="junk2_ps")
        for _f in range(30):
            nc.tensor.matmul(junk2_ps[:], kT_sb[:, 0, :128], kT_sb[:, 1, :512],
                             start=True, stop=True)

        # ---------------- Stage 7: W_O ------------------------------------
        oT_sb = attn.tile([128, H, TLOC], BF, name="oT_sb")
        _oTv = oT_sb[:].rearrange("p (s hh) t -> p s hh t", s=NCORES)
        _a2aov = a2ao_out[:, :, :, :].rearrange("s hh v t -> v s hh t")
        for _q, _eng in ((0, nc.gpsimd), (1, nc.scalar), (2, nc.gpsimd),
                         (3, nc.scalar)):
            nc_s = slice(_q * 2, _q * 2 + 2)
            _eng.dma_start(out=_oTv[:, nc_s], in_=_a2aov[:, nc_s])
        outp_cm = tc.tile_pool(name="outp", bufs=4)
        outp = outp_cm.__enter__()
        for ht in range(NHT):
            pso = [ps_mm.tile([128, 512], F32, tag="mm", name="wo_ps")
                   for _ in range(MCH)]
            for c in range(H):
                wt = wo_tiles[2 * ht + c // 8]
                for m in range(MCH):
                    nc.tensor.matmul(pso[m][:], oT_sb[:, c, m * 128:(m + 1) * 128],
                                     wt[:, (c % 8) * 512:(c % 8 + 1) * 512],
                                     start=(c == 0), stop=(c == H - 1))
            for m in range(MCH):
                ot = outp.tile([128, 512], F32, tag="ot", name="ot")
                if ht % 2 == 0:
                    nc.scalar.copy(ot[:], pso[m][:])
                else:
                    nc.vector.tensor_copy(ot[:], pso[m][:])
                nc.scalar.dma_start(
                    out=out[:, :].rearrange("(m p) d -> p m d", p=128)[
                        :, m, ht * 512:(ht + 1) * 512],
                    in_=ot[:])

        for p in (outp_cm, onorm_cm, pexp_cm, ps_o_cm, attn_cm, wo_cm,
                  ps_tr_cm, ps_mm_cm, wstream_cm, dram_cm, consts_cm):
            p.__exit__(None, None, None)

    nc.finalize()
    return nc


def _to_bf16(a):
    return np.asarray(a, dtype=np.float32).astype(ml_dtypes.bfloat16)


def _prep_in_maps(positions, hidden_states, w_fused, w_qb, w_kvb, w_o,
                  qa_ln_w, kva_ln_w):
    positions = np.asarray(positions)
    hidden_states = np.asarray(hidden_states, dtype=np.float32)
    w_fused = np.asarray(w_fused, dtype=np.float32)
    w_qb = np.asarray(w_qb, dtype=np.float32)
    w_kvb = np.asarray(w_kvb, dtype=np.float32)
    w_o = np.asarray(w_o, dtype=np.float32)
    qa_ln_w = np.asarray(qa_ln_w, dtype=np.float32)
    kva_ln_w = np.asarray(kva_ln_w, dtype=np.float32)

    inv_freq = 1.0 / (THETA ** (np.arange(0, DR, 2, dtype=np.float32) / DR))
    freqs = positions.astype(np.float32)[:, None] * inv_freq[None, :]
    cs_full = np.concatenate([np.cos(freqs), np.sin(freqs)], axis=1)  # [T, 64]

    wqb_folded = qa_ln_w[:, None] * w_qb
    wkvb_r = w_kvb.reshape(KVL, H, DN + DV)

    # wf kv+pe tiles: variable k-chunks per tile, rows contiguous per tile
    kvpe_cols = w_fused[:, QL:QL + KVPE]                       # [5120, 576]
    _blocks = []
    _k0 = 0
    for _nk in WF_KV_TILES:
        _blocks.append(
            kvpe_cols[_k0 * 128:(_k0 + _nk) * 128]
            .reshape(_nk, 128, KVPE).transpose(1, 0, 2).reshape(-1))
        _k0 += _nk
    wf_kv_arr = _to_bf16(np.concatenate(_blocks))
    # wf q tiles: [20][128][2*1536]
    q_cols = w_fused[:, :QL]                                   # [5120, 1536]
    wf_q_arr = _to_bf16(
        q_cols.reshape(NT_Q, 2, 128, QL).transpose(0, 2, 1, 3)
              .reshape(NT_Q, 128, 2 * QL))
    # wqb tiles: [half][k][128][1536]
    wqb_arr = _to_bf16(
        wqb_folded.reshape(QKD, 128, 2, QL).transpose(2, 0, 1, 3))
    wqb_arr = np.ascontiguousarray(wqb_arr)
    # wo tiles: [2*ht + c//8][p][(c%8)*512+j] = w_o[c*128+p, ht*512+j]
    wo_arr = _to_bf16(
        w_o.reshape(2, H // 2, 128, NHT, 512).transpose(3, 0, 2, 1, 4)
           .reshape(2 * NHT, 128, (H // 2) * 512))
    wo_arr = np.ascontiguousarray(wo_arr)

    tri = np.triu(np.ones((128, 128), np.float32))
    cmask = _to_bf16(np.repeat(tri[:, None, :], HLOC, axis=1))

    in_maps = []
    for c in range(NCORES):
        tok = slice(c * TLOC, (c + 1) * TLOC)
        heads = [HLOC * c + i for i in range(HLOC)]
        wkcT = np.stack([(wkvb_r[:, h, :DN] * kva_ln_w[:, None]).T for h in heads])
        wvc = np.concatenate(
            [wkvb_r[:, h, DN:] * kva_ln_w[:, None] for h in heads], axis=1)
        hT_arr = _to_bf16(np.ascontiguousarray(
            hidden_states[tok].T.reshape(KD, 128, TLOC).transpose(1, 0, 2)
                              .reshape(128, KD * TLOC)))
        in_maps.append({
            "hT": hT_arr,
            "wf_kv": wf_kv_arr,
            "wf_q": wf_q_arr,
            "wqb": wqb_arr,
            "cs": np.ascontiguousarray(cs_full[tok]),
            "wkcT": _to_bf16(np.ascontiguousarray(wkcT)),
            "wvc": _to_bf16(np.ascontiguousarray(wvc)),
            "wo": wo_arr,
            "cmask": cmask,
        })
    return in_maps


def kernel(**inputs):
    global _NC_CACHE, _last_in_maps
    in_maps = _prep_in_maps(**inputs)
    _last_in_maps = in_maps
    if _NC_CACHE is None:
        _NC_CACHE = build_nc()

    res = run_bass_kernel_spmd(_NC_CACHE, in_maps, core_ids=list(range(NCORES)))
    return np.concatenate([np.asarray(res.results[c]["out"], dtype=np.float32)
                           for c in range(NCORES)], axis=0)


if __name__ == "__main__":
    build_nc()
    print("build ok")
